# revision 1
# baseline (speedup 1.0000x reference)
"""BAD-descriptor kernel for Trainium2 (8 NeuronCores, SPMD over pairs).

Math: the reference gathers from an integral image at
  cy = clip(h + off_y, 0, H-1).astype(int) + r,  y0/y1 = cy -/+ rad(+1)
Because h is an integer grid, clip(h+off).astype(int) == clip(h + floor(off), 0, H-1),
so each box-mean term is just the radius-d box-mean image sampled at a clamped
integer 2D shift.  With only 3 radii we precompute, per batch b and d in {1,2,3},
the box-mean image BM_d (edge-replicate semantics of the reference integral image),
pad it by 16 with edge replication into BMP_d [256,256], and then

  out[b,p] = BMP_{d_p}[b][sy1:sy1+224, sx1:sx1+224]
           - BMP_{d_p}[b][sy2:sy2+224, sx2:sx2+224] - thr_p,
  sy = floor(off_y)+16 in [0,32], sx likewise.

Per-core device program (32 pairs/core):
  A) pair prep: floor/clip arithmetic on the offset vectors (DVE), producing
     int32 row/col window offsets in SBUF + negated thresholds broadcast
     across partitions.
  B) box-mean precompute: horizontal (2d+1)-taps via DVE shifted adds on
     column-padded x, vertical taps via PE matmul with constant band matrices
     (passed as input constants), scaled 1/area on ACT, column/row replicate
     padding, DMA into a DRAM scratch bmp[2,768,256].
  C) main loop over (p, b): two dynamic-offset HWDGE window DMAs (registers
     loaded from SBUF with values_load), one fused DVE op
     (W1 + (-thr)) - W2, one DMA to the output.
"""

import sys

sys.path.insert(0, "/opt/trn_rl_repo")

import numpy as np

import concourse.bass as bass
import concourse.bacc as bacc
import concourse.mybir as mybir
import concourse.tile as tile
from concourse.bass_utils import run_bass_kernel_spmd

B = 2
H = W = 224
P_TOTAL = 256
N_CORES = 8
P_CORE = P_TOTAL // N_CORES  # 32
PAD = 16
RMAX = 3
HP = H + 2 * PAD  # 256 padded image rows
F32 = mybir.dt.float32
I32 = mybir.dt.int32

# window tile: 2 image rows per partition -> [112, 448] ([112, 2, 224] view)
NPART = 112
NFREE = (H * W) // NPART  # 448


def _band_matrices() -> np.ndarray:
    """Vertical band matrices with the +-16 replicate pad baked in.

    sdt[0][r, d-1, m]: hs-tile0 row r (x rows 0..127) -> BMP block row m
        (m in [0,128): h = max(m-16, 0)).
    sdt[1][k, d-1, m]: hs-tile1 row 96+k -> BMP block row 128+m
        (h = min(112+m, 223)).
    entry = #{i in [-d,d] : clip(h+i, 0, H-1) == row}.
    """
    sdt = np.zeros((2, 128, 3, 128), np.float32)
    for d in (1, 2, 3):
        for m in range(128):
            h_lo = max(m - PAD, 0)
            h_hi = min(112 + m, H - 1)
            for i in range(-d, d + 1):
                r = min(max(h_lo + i, 0), H - 1)
                if r < 128:
                    sdt[0][r, d - 1, m] += 1.0
                r = min(max(h_hi + i, 0), H - 1)
                if 96 <= r:
                    sdt[1][r - 96, d - 1, m] += 1.0
    return sdt


def build_device_program(nc: bacc.Bacc):
    x_ap = nc.dram_tensor("x", [B, H, W], F32, kind="ExternalInput").ap()
    # rows: offy1, offx1, offy2, offx2, thr
    vecs_ap = nc.dram_tensor("vecs", [5, P_CORE], F32, kind="ExternalInput").ap()
    radii_ap = nc.dram_tensor("radii", [1, P_CORE], I32, kind="ExternalInput").ap()
    sdt_ap = nc.dram_tensor("sdt", [2, 128, 3, 128], F32, kind="ExternalInput").ap()
    # batch-interleaved output [p, h, b, w]; host un-interleaves
    out_ap = nc.dram_tensor("out", [P_CORE, H, B, W], F32, kind="ExternalOutput").ap()

    with tile.TileContext(nc) as tc:
        build_kernel(tc, out_ap, x_ap, vecs_ap, radii_ap, sdt_ap)
    return nc


def build_kernel(tc, out_ap, x_ap, vecs_ap, radii_ap, sdt_ap):
    nc = tc.nc
    EngT = mybir.EngineType
    Alu = mybir.AluOpType
    Act = mybir.ActivationFunctionType

    from contextlib import ExitStack
    ctx = ExitStack()
    const_pool = ctx.enter_context(tc.tile_pool(name="const", bufs=1))
    work_pool = ctx.enter_context(tc.tile_pool(name="work", bufs=1))
    psum_pool = ctx.enter_context(tc.tile_pool(name="psum", bufs=4, space="PSUM"))
    dram_pool = ctx.enter_context(tc.tile_pool(name="dram", bufs=1, space="DRAM"))
    slab_pool = ctx.enter_context(tc.tile_pool(name="slab", bufs=8))
    o_pool = ctx.enter_context(tc.tile_pool(name="outt", bufs=6))

    # ---------------- Stage A: pair prep ----------------
    # one DMA for the five fp32 vectors, one for radii
    vt = const_pool.tile([1, 5, P_CORE], F32, tag="v_all")
    nc.scalar.dma_start(out=vt[:], in_=vecs_ap[:])
    vecs = {name: vt[0:1, i, :] for i, name in enumerate(
        ("offy1", "offx1", "offy2", "offx2", "thr"))}
    radii_t = const_pool.tile([1, P_CORE], I32, tag="v_radii")
    nc.scalar.dma_start(out=radii_t[:], in_=radii_ap[:])

    radf = const_pool.tile([1, P_CORE], F32, tag="radf")
    nc.vector.tensor_copy(out=radf[:], in_=radii_t[:])
    # clamp radius to [1,3] for safety
    nc.vector.tensor_scalar(out=radf[:], in0=radf[:], scalar1=1.0, scalar2=3.0,
                            op0=Alu.max, op1=Alu.min)

    def floor_to_base(off_t, name):
        """return [1,P_CORE] f32 tile with clip(floor(off),-16,16)+16 in [0,32]."""
        ti = const_pool.tile([1, P_CORE], I32, tag=f"fi_{name}")
        tf = const_pool.tile([1, P_CORE], F32, tag=f"ff_{name}")
        gt = const_pool.tile([1, P_CORE], F32, tag=f"gt_{name}")
        res = const_pool.tile([1, P_CORE], F32, tag=f"fl_{name}")
        nc.vector.tensor_copy(out=ti[:], in_=off_t[:])   # cast (round or trunc)
        nc.vector.tensor_copy(out=tf[:], in_=ti[:])      # back to f32, exact
        nc.vector.tensor_tensor(out=gt[:], in0=tf[:], in1=off_t[:], op=Alu.is_gt)
        nc.vector.tensor_tensor(out=res[:], in0=tf[:], in1=gt[:], op=Alu.subtract)
        # + PAD then clamp to [0, 2*PAD]
        nc.vector.tensor_scalar_add(out=res[:], in0=res[:], scalar1=float(PAD))
        nc.vector.tensor_scalar(out=res[:], in0=res[:], scalar1=0.0,
                                scalar2=float(2 * PAD), op0=Alu.max, op1=Alu.min)
        return res

    sy1 = floor_to_base(vecs["offy1"], "y1")
    sx1 = floor_to_base(vecs["offx1"], "x1")
    sy2 = floor_to_base(vecs["offy2"], "y2")
    sx2 = floor_to_base(vecs["offx2"], "x2")

    # flat element offset into interleaved bmp: ((d-1)*HP + sy)*2*HP + sx
    dbase = const_pool.tile([1, P_CORE], F32, tag="dbase")
    nc.vector.tensor_scalar(out=dbase[:], in0=radf[:], scalar1=1.0, scalar2=float(HP),
                            op0=Alu.subtract, op1=Alu.mult)
    off1 = const_pool.tile([1, P_CORE], I32, tag="off1")
    off2 = const_pool.tile([1, P_CORE], I32, tag="off2")
    for sy, sx, off, nm in ((sy1, sx1, off1, "1"), (sy2, sx2, off2, "2")):
        rowf = const_pool.tile([1, P_CORE], F32, tag=f"rowf{nm}")
        nc.vector.tensor_tensor(out=rowf[:], in0=dbase[:], in1=sy[:], op=Alu.add)
        nc.vector.tensor_scalar_mul(out=rowf[:], in0=rowf[:], scalar1=float(B * HP))
        nc.vector.tensor_tensor(out=rowf[:], in0=rowf[:], in1=sx[:], op=Alu.add)
        nc.vector.tensor_copy(out=off[:], in_=rowf[:])

    # thresholds broadcast to all partitions via a step-0 DMA from DRAM
    thr_bc = const_pool.tile([NPART, P_CORE], F32, tag="thr_bc")
    nc.scalar.dma_start(out=thr_bc[:],
                        in_=vecs_ap[4:5, :].to_broadcast((NPART, P_CORE)))

    # ---------------- Stage B: box-mean precompute ----------------
    # bmp scratch in DRAM, batch-interleaved by row: [3*HP, B, HP]
    bmp = dram_pool.tile([3 * HP, B, HP], F32, tag="bmp")

    part_rows = ((0, 128), (96, 128))  # (row0, nrows) x-row tiles (overlapping)

    # x tiles carry both batches side by side in the free dim: [nr, 2, 230];
    # the matmul N-dim and all stage-B ops then cover both batches at once.
    xts = []
    for j, (r0, nr) in enumerate(part_rows):
        xt = work_pool.tile([nr, B, W + 2 * RMAX], F32, tag=f"xt_{j}")
        for b in range(B):
            eng = nc.sync if b == 0 else nc.scalar
            eng.dma_start(out=xt[:, b, RMAX:RMAX + W], in_=x_ap[b, r0:r0 + nr, :])
        nc.vector.tensor_copy(
            out=xt[:, :, 0:RMAX],
            in_=xt[:, :, RMAX:RMAX + 1].to_broadcast((nr, B, RMAX)))
        nc.vector.tensor_copy(
            out=xt[:, :, RMAX + W:],
            in_=xt[:, :, RMAX + W - 1:RMAX + W].to_broadcast((nr, B, RMAX)))
        xts.append(xt)

    # Band constants with the replicate pads baked in (see _band_matrices):
    # each d-block needs exactly two [K=128, M=128, N=448] matmuls. Loaded
    # after x so the x DMAs (which gate the hs chain) go out first.
    sdt_lo = const_pool.tile([128, 3, 128], F32, tag="sdt_lo")
    sdt_hi = const_pool.tile([128, 3, 128], F32, tag="sdt_hi")
    nc.sync.dma_start(out=sdt_lo[:], in_=sdt_ap[0])
    nc.scalar.dma_start(out=sdt_hi[:], in_=sdt_ap[1])

    # horizontal box sums hs[d][j]: [nr, B, W]
    hs = {1: [], 2: [], 3: []}
    for j, (r0, nr) in enumerate(part_rows):
        xt = xts[j]
        eng = nc.vector
        h1 = work_pool.tile([nr, B, W], F32, tag=f"hs1_{j}")
        h2 = work_pool.tile([nr, B, W], F32, tag=f"hs2_{j}")
        h3 = work_pool.tile([nr, B, W], F32, tag=f"hs3_{j}")
        ta = work_pool.tile([nr, B, W], F32, tag=f"hta_{j}")
        sl = lambda c: xt[:, :, c:c + W]
        eng.tensor_tensor(out=ta[:], in0=sl(2), in1=sl(3), op=Alu.add)
        eng.tensor_tensor(out=h1[:], in0=ta[:], in1=sl(4), op=Alu.add)
        eng.tensor_tensor(out=ta[:], in0=sl(1), in1=sl(5), op=Alu.add)
        eng.tensor_tensor(out=h2[:], in0=h1[:], in1=ta[:], op=Alu.add)
        eng.tensor_tensor(out=ta[:], in0=sl(0), in1=sl(6), op=Alu.add)
        eng.tensor_tensor(out=h3[:], in0=h2[:], in1=ta[:], op=Alu.add)
        hs[1].append(h1)
        hs[2].append(h2)
        hs[3].append(h3)

    for d in (1, 2, 3):
        area = float((2 * d + 1) ** 2)
        dr0 = (d - 1) * HP  # row-block base of this d in bmp
        NB = B * W  # matmul N covers both batches (448 <= 512 fp32 limit)
        for j in range(2):
            ps = psum_pool.tile([128, NB], F32, tag=f"ps{j}")
            sdt_t = sdt_lo if j == 0 else sdt_hi
            nc.tensor.matmul(out=ps[:], lhsT=sdt_t[:, d - 1, :],
                             rhs=hs[d][j][:].rearrange("r b w -> r (b w)"),
                             start=True, stop=True)
            # scale + column pads -> bmc [128, B, HP] (BMP rows incl row pads)
            bmc = work_pool.tile([128, B, HP], F32, tag=f"bmc_{d}_{j}")
            nc.scalar.activation(bmc[:, :, PAD:PAD + W],
                                 ps[:].rearrange("r (b w) -> r b w", b=B),
                                 Act.Copy, scale=1.0 / area)
            nc.vector.tensor_copy(
                out=bmc[:, :, 0:PAD],
                in_=bmc[:, :, PAD:PAD + 1].to_broadcast((128, B, PAD)))
            nc.vector.tensor_copy(
                out=bmc[:, :, PAD + W:],
                in_=bmc[:, :, PAD + W - 1:PAD + W].to_broadcast((128, B, PAD)))
            eng = nc.sync if j == 0 else nc.scalar
            eng.dma_start(
                out=bmp[dr0 + 128 * j: dr0 + 128 * (j + 1), :, :].rearrange(
                    "r b w -> (r b) w"),
                in_=bmc[:])

    # ---------------- Stage C: main loop ----------------
    # The row-interleaved bmp layout makes one window for BOTH batches a
    # single 2D AP: 448 rows (b0/b1 alternating), row stride HP, width 224.
    # Lands in [112, 896]: partition k = rows (h=2k..2k+1) x (b0,b1), i.e.
    # flat (h, b, w) order — matching the interleaved out layout [p, h, b, w].
    bmp_full = bmp[:, :, :]
    bmp_base = bmp_full.offset
    assert isinstance(bmp_base, int)
    MAXOFF = (3 * HP - H) * B * HP  # conservative bound for offsets

    def slab_src(offv):
        return bass.AP(bmp_full.tensor, offv + bmp_base,
                       [[HP, B * H], [1, W]])

    for p in range(P_CORE):
        o1v = nc.values_load(off1[0:1, p:p + 1], engines=[EngT.Activation],
                             min_val=0, max_val=MAXOFF,
                             skip_runtime_bounds_check=True)
        o2v = nc.values_load(off2[0:1, p:p + 1], engines=[EngT.SP],
                             min_val=0, max_val=MAXOFF,
                             skip_runtime_bounds_check=True)
        s1 = slab_pool.tile([NPART, 2 * NFREE], F32, tag="s1")
        s2 = slab_pool.tile([NPART, 2 * NFREE], F32, tag="s2")
        nc.scalar.dma_start(out=s1[:], in_=slab_src(o1v))
        nc.sync.dma_start(out=s2[:], in_=slab_src(o2v))
        o = o_pool.tile([NPART, 2 * NFREE], F32, tag="o")
        nc.vector.scalar_tensor_tensor(out=o[:], in0=s1[:],
                                       scalar=thr_bc[0:NPART, p:p + 1], in1=s2[:],
                                       op0=Alu.subtract, op1=Alu.subtract)
        nc.sync.dma_start(out=out_ap[p].rearrange("h b w -> (h b) w"),
                          in_=o[:].rearrange("k (j w) -> k j w", j=4))

    ctx.close()


_COMPILED = {}


def _get_compiled():
    if "nc" not in _COMPILED:
        nc = bacc.Bacc("TRN2", target_bir_lowering=False, debug=False,
                       num_devices=N_CORES)
        build_device_program(nc)
        nc.compile()
        _COMPILED["nc"] = nc
    return _COMPILED["nc"]


def _ensure_ntff_hook():
    """The agent image's antenv lacks axon_hooks; shim it so trace=True can
    drive NTFF profiling via the boot module's ctypes hook (test-only path)."""
    import types

    try:
        from antenv.axon_hooks import get_axon_ntff_profile_hook  # noqa: F401
        return
    except ImportError:
        pass
    import antenv

    mod = types.ModuleType("antenv.axon_hooks")
    _hook = [None]
    mod.set_axon_ntff_profile_hook = lambda h: _hook.__setitem__(0, h)
    mod.get_axon_ntff_profile_hook = lambda: _hook[0]
    sys.modules["antenv.axon_hooks"] = mod
    antenv.axon_hooks = mod
    from trn_agent_boot.trn_boot import _ntff_profile_via_ctypes

    mod.set_axon_ntff_profile_hook(
        _ntff_profile_via_ctypes("/opt/axon/libaxon_pjrt.so"))


def run(inputs: dict, trace: bool = False):
    """Run on the 8 cores. Returns (full output [B,256,H,W], exec_time_ns|None)."""
    x = np.asarray(inputs["x"], dtype=np.float32).reshape(B, H, W)
    offset_x1 = np.asarray(inputs["offset_x1"], np.float32)
    offset_x2 = np.asarray(inputs["offset_x2"], np.float32)
    offset_y1 = np.asarray(inputs["offset_y1"], np.float32)
    offset_y2 = np.asarray(inputs["offset_y2"], np.float32)
    radii = np.asarray(inputs["radii"]).astype(np.int32)
    thresholds = np.asarray(inputs["thresholds"], np.float32)

    sdt = _band_matrices()
    nc = _get_compiled()

    in_maps = []
    for c in range(N_CORES):
        sl = slice(c * P_CORE, (c + 1) * P_CORE)
        vecs = np.stack([offset_y1[sl], offset_x1[sl], offset_y2[sl],
                         offset_x2[sl], thresholds[sl]]).astype(np.float32)
        in_maps.append({
            "x": x,
            "vecs": vecs,
            "radii": radii[sl].reshape(1, P_CORE),
            "sdt": sdt,
        })

    if trace:
        _ensure_ntff_hook()
    res = run_bass_kernel_spmd(nc, in_maps, list(range(N_CORES)), trace=trace)
    # per-core out is [P_CORE, H, B, W]; un-interleave to [B, P_TOTAL, H, W]
    allc = np.stack([res.results[c]["out"] for c in range(N_CORES)])
    full = np.ascontiguousarray(allc.transpose(3, 0, 1, 2, 4)).reshape(
        B, P_TOTAL, H, W)
    return full, res.exec_time_ns


def kernel(x, offset_x1, offset_x2, offset_y1, offset_y2, radii, thresholds,
           max_radius):
    out, _ = run({
        "x": x, "offset_x1": offset_x1, "offset_x2": offset_x2,
        "offset_y1": offset_y1, "offset_y2": offset_y2,
        "radii": radii, "thresholds": thresholds, "max_radius": max_radius,
    })
    return out


if __name__ == "__main__":
    # smoke test with random data
    rng = np.random.default_rng(0)
    out = kernel(
        x=rng.standard_normal((B, 1, H, W), dtype=np.float32),
        offset_x1=rng.uniform(-16, 16, P_TOTAL).astype(np.float32),
        offset_x2=rng.uniform(-16, 16, P_TOTAL).astype(np.float32),
        offset_y1=rng.uniform(-16, 16, P_TOTAL).astype(np.float32),
        offset_y2=rng.uniform(-16, 16, P_TOTAL).astype(np.float32),
        radii=rng.integers(1, 4, P_TOTAL).astype(np.int32),
        thresholds=(rng.standard_normal(P_TOTAL) * 0.1).astype(np.float32),
        max_radius=3,
    )
    print("out", out.shape, out.dtype, float(np.abs(out).max()))



# revision 8
# speedup vs baseline: 1.5068x; 1.5068x over previous
"""BAD-descriptor kernel for Trainium2 (8 NeuronCores, SPMD over pairs).

Math: the reference gathers from an integral image at
  cy = clip(h + off_y, 0, H-1).astype(int) + r,  y0/y1 = cy -/+ rad(+1)
Because h is an integer grid, clip(h+off).astype(int) == clip(h + floor(off), 0, H-1),
so each box-mean term is the radius-d box-mean image sampled at a clamped
integer 2D shift.  With only 3 radii we precompute, per batch b and d in {1,2,3},
the box-mean image BM_d (edge-replicate semantics of the reference integral
image), padded by 16 with edge replication into BMP_d [256,256]:

  out[b,p] = BMP_{d_p}[b][sy1:sy1+224, sx1:sx1+224]
           - BMP_{d_p}[b][sy2:sy2+224, sx2:sx2+224] - thr_p,
  sy = floor(off_y)+16 in [0,32], sx likewise.

v2 (PE-gather): the 2D-shifted window read is done on the TENSOR engine, not
DMA, so the only HBM traffic is the input image and the (fp16) output.

  out[m, n] = sum_k E[k, m] * BMP[k, sx + n]   with  E[k, m] = d(k == m + sy)

E slices come from one constant +-identity matrix indexed with a dynamic
free-dim offset (values_load registers), BMP row-blocks live in SBUF as fp16
planes [128, 3, 2, 256], and the W1 - W2 subtraction happens for free via
PSUM accumulation of +E and -E matmuls.  Per pair: 6 matmuls (K=128, N=448)
-> psum0 [128,448] rows 0..127, psum1 [96,448] rows 128..223; ACT evicts with
bias=-thr to fp16 staging; one output DMA per 8 pairs.
"""

import sys

sys.path.insert(0, "/opt/trn_rl_repo")

import numpy as np

import concourse.bass as bass
import concourse.bacc as bacc
import concourse.mybir as mybir
import concourse.tile as tile
from concourse.bass_utils import run_bass_kernel_spmd

B = 2
H = W = 224
P_TOTAL = 256
N_CORES = 8
P_CORE = P_TOTAL // N_CORES  # 32
PAD = 16
RMAX = 3
HP = H + 2 * PAD  # 256 padded image rows/cols
F32 = mybir.dt.float32
F16 = mybir.dt.float16
I32 = mybir.dt.int32

NB = B * W        # 448 matmul N (b, w)
GRP = 8           # pairs per output DMA
EW = 384          # identity block width (j dim) per sign


def _band_matrices() -> np.ndarray:
    """Vertical band matrices with the +-16 replicate pad baked in.

    sdt[0][r, d-1, m]: hs-tile0 row r (x rows 0..127) -> BMP block row m
        (m in [0,128): h = max(m-16, 0)).
    sdt[1][k, d-1, m]: hs-tile1 row 96+k -> BMP block row 128+m
        (h = min(112+m, 223)).
    entry = #{i in [-d,d] : clip(h+i, 0, H-1) == row}.
    """
    sdt = np.zeros((2, 128, 3, 128), np.float16)
    for d in (1, 2, 3):
        for m in range(128):
            h_lo = max(m - PAD, 0)
            h_hi = min(112 + m, H - 1)
            for i in range(-d, d + 1):
                r = min(max(h_lo + i, 0), H - 1)
                if r < 128:
                    sdt[0][r, d - 1, m] += 1.0
                r = min(max(h_hi + i, 0), H - 1)
                if 96 <= r:
                    sdt[1][r - 96, d - 1, m] += 1.0
    return sdt


def _shift_identity() -> np.ndarray:
    """e2 [128, 2*EW] fp16: e2[k, j] = d(k == j-128), e2[k, EW+j] = -d(k == j-128)."""
    e = np.zeros((128, 2 * EW), np.float16)
    for k in range(128):
        e[k, 128 + k] = 1.0
        e[k, EW + 128 + k] = -1.0
    return e


def build_device_program(nc: bacc.Bacc):
    x_ap = nc.dram_tensor("x", [B, H, W], F32, kind="ExternalInput").ap()
    # rows: offy1, offx1, offy2, offx2, NEGATED thr
    vecs_ap = nc.dram_tensor("vecs", [5, P_CORE], F32, kind="ExternalInput").ap()
    radii_ap = nc.dram_tensor("radii", [1, P_CORE], I32, kind="ExternalInput").ap()
    sdt_ap = nc.dram_tensor("sdt", [2, 128, 3, 128], F16, kind="ExternalInput").ap()
    e2_ap = nc.dram_tensor("e2", [128, 2 * EW], F16, kind="ExternalInput").ap()
    # fp16 outputs: block0 rows 0..127, block1 rows 128..223; [hpart, p, b, w]
    out0_ap = nc.dram_tensor("out0", [128, P_CORE, B, W], F16,
                             kind="ExternalOutput").ap()
    out1_ap = nc.dram_tensor("out1", [96, P_CORE, B, W], F16,
                             kind="ExternalOutput").ap()

    with tile.TileContext(nc) as tc:
        build_kernel(tc, out0_ap, out1_ap, x_ap, vecs_ap, radii_ap, sdt_ap, e2_ap)
    return nc


def build_kernel(tc, out0_ap, out1_ap, x_ap, vecs_ap, radii_ap, sdt_ap, e2_ap):
    nc = tc.nc
    EngT = mybir.EngineType
    Alu = mybir.AluOpType
    Act = mybir.ActivationFunctionType

    from contextlib import ExitStack
    ctx = ExitStack()
    const_pool = ctx.enter_context(tc.tile_pool(name="const", bufs=1))
    work_pool = ctx.enter_context(tc.tile_pool(name="work", bufs=1))
    psumB_pool = ctx.enter_context(tc.tile_pool(name="psumB", bufs=1, space="PSUM"))
    psum_pool = ctx.enter_context(tc.tile_pool(name="psum", bufs=3, space="PSUM"))
    stage_pool = ctx.enter_context(tc.tile_pool(name="stage", bufs=2))

    # ---------------- Stage A: pair prep ----------------
    vt = const_pool.tile([1, 5, P_CORE], F32, tag="v_all")
    nc.scalar.dma_start(out=vt[:], in_=vecs_ap[:])
    vecs = {name: vt[0:1, i, :] for i, name in enumerate(
        ("offy1", "offx1", "offy2", "offx2", "negthr"))}
    radii_t = const_pool.tile([1, P_CORE], I32, tag="v_radii")
    nc.scalar.dma_start(out=radii_t[:], in_=radii_ap[:])

    radf = const_pool.tile([1, P_CORE], F32, tag="radf")
    nc.vector.tensor_copy(out=radf[:], in_=radii_t[:])
    nc.vector.tensor_scalar(out=radf[:], in0=radf[:], scalar1=1.0, scalar2=3.0,
                            op0=Alu.max, op1=Alu.min)

    def floor_to_base(off_t, name):
        """return [1,P_CORE] f32 tile with clip(floor(off),-16,16)+16 in [0,32]."""
        ti = const_pool.tile([1, P_CORE], I32, tag=f"fi_{name}")
        tf = const_pool.tile([1, P_CORE], F32, tag=f"ff_{name}")
        gt = const_pool.tile([1, P_CORE], F32, tag=f"gt_{name}")
        res = const_pool.tile([1, P_CORE], F32, tag=f"fl_{name}")
        nc.vector.tensor_copy(out=ti[:], in_=off_t[:])   # cast (round or trunc)
        nc.vector.tensor_copy(out=tf[:], in_=ti[:])      # back to f32, exact
        nc.vector.tensor_tensor(out=gt[:], in0=tf[:], in1=off_t[:], op=Alu.is_gt)
        nc.vector.tensor_tensor(out=res[:], in0=tf[:], in1=gt[:], op=Alu.subtract)
        nc.vector.tensor_scalar_add(out=res[:], in0=res[:], scalar1=float(PAD))
        nc.vector.tensor_scalar(out=res[:], in0=res[:], scalar1=0.0,
                                scalar2=float(2 * PAD), op0=Alu.max, op1=Alu.min)
        return res

    sy1 = floor_to_base(vecs["offy1"], "y1")
    sx1 = floor_to_base(vecs["offx1"], "x1")
    sy2 = floor_to_base(vecs["offy2"], "y2")
    sx2 = floor_to_base(vecs["offx2"], "x2")

    # per-pair register table [1, P_CORE, 6]:
    #   0: (d-1)*512 + sx1   (rhs offset, window 1; PE regs)
    #   1: (d-1)*512 + sx2   (rhs offset, window 2; PE regs)
    #   E-slice offsets into e2t (DVE regs for the lhsT staging copies):
    #   2: 128+sy1   3: sy1   4: EW+128+sy2   5: EW+sy2
    tab_f = const_pool.tile([1, P_CORE, 6], F32, tag="tab_f")
    dbase = const_pool.tile([1, P_CORE], F32, tag="dbase")
    nc.vector.tensor_scalar(out=dbase[:], in0=radf[:], scalar1=1.0, scalar2=512.0,
                            op0=Alu.subtract, op1=Alu.mult)
    nc.vector.tensor_tensor(out=tab_f[0:1, :, 0], in0=dbase[:], in1=sx1[:], op=Alu.add)
    nc.vector.tensor_tensor(out=tab_f[0:1, :, 1], in0=dbase[:], in1=sx2[:], op=Alu.add)
    nc.vector.tensor_scalar_add(out=tab_f[0:1, :, 2], in0=sy1[:], scalar1=128.0)
    nc.vector.tensor_copy(out=tab_f[0:1, :, 3], in_=sy1[:])
    nc.vector.tensor_scalar_add(out=tab_f[0:1, :, 4], in0=sy2[:],
                                scalar1=float(EW + 128))
    nc.vector.tensor_scalar_add(out=tab_f[0:1, :, 5], in0=sy2[:], scalar1=float(EW))
    tab_i = const_pool.tile([1, P_CORE, 6], I32, tag="tab_i")
    nc.vector.tensor_copy(out=tab_i[:], in_=tab_f[:])

    # negated thresholds broadcast to 128 partitions (ACT bias, fp32)
    negthr = const_pool.tile([128, P_CORE], F32, tag="negthr")
    nc.scalar.dma_start(out=negthr[:],
                        in_=vecs_ap[4:5, :].to_broadcast((128, P_CORE)))

    # ---------------- Stage B: box-mean planes in SBUF (fp16) ----------------
    part_rows = ((0, 128), (96, 128))  # (row0, nrows) x-row tiles (overlapping)

    xts = []
    for j, (r0, nr) in enumerate(part_rows):
        xt = work_pool.tile([nr, B, W + 2 * RMAX], F32, tag=f"xt_{j}")
        for b in range(B):
            eng = nc.sync if b == 0 else nc.scalar
            eng.dma_start(out=xt[:, b, RMAX:RMAX + W], in_=x_ap[b, r0:r0 + nr, :])
        nc.vector.tensor_copy(
            out=xt[:, :, 0:RMAX],
            in_=xt[:, :, RMAX:RMAX + 1].to_broadcast((nr, B, RMAX)))
        nc.vector.tensor_copy(
            out=xt[:, :, RMAX + W:],
            in_=xt[:, :, RMAX + W - 1:RMAX + W].to_broadcast((nr, B, RMAX)))
        xts.append(xt)

    sdt_lo = const_pool.tile([128, 3, 128], F16, tag="sdt_lo")
    sdt_hi = const_pool.tile([128, 3, 128], F16, tag="sdt_hi")
    nc.sync.dma_start(out=sdt_lo[:], in_=sdt_ap[0])
    nc.scalar.dma_start(out=sdt_hi[:], in_=sdt_ap[1])
    e2t = const_pool.tile([128, 2 * EW], F16, tag="e2t")
    nc.sync.dma_start(out=e2t[:], in_=e2_ap[:])

    # horizontal box sums hs[d][j]: [nr, B, W] fp16
    hs = {1: [], 2: [], 3: []}
    for j, (r0, nr) in enumerate(part_rows):
        xt = xts[j]
        eng = nc.vector
        h1 = work_pool.tile([nr, B, W], F16, tag=f"hs1_{j}")
        h2 = work_pool.tile([nr, B, W], F16, tag=f"hs2_{j}")
        h3 = work_pool.tile([nr, B, W], F16, tag=f"hs3_{j}")
        ta = work_pool.tile([nr, B, W], F32, tag=f"hta_{j}")
        tb = work_pool.tile([nr, B, W], F32, tag=f"htb_{j}")
        sl = lambda c: xt[:, :, c:c + W]
        eng.tensor_tensor(out=ta[:], in0=sl(2), in1=sl(3), op=Alu.add)
        eng.tensor_tensor(out=h1[:], in0=ta[:], in1=sl(4), op=Alu.add)
        eng.tensor_tensor(out=tb[:], in0=sl(1), in1=sl(5), op=Alu.add)
        eng.tensor_tensor(out=ta[:], in0=ta[:], in1=sl(4), op=Alu.add)  # = h1 f32
        eng.tensor_tensor(out=h2[:], in0=ta[:], in1=tb[:], op=Alu.add)
        eng.tensor_tensor(out=ta[:], in0=ta[:], in1=tb[:], op=Alu.add)  # = h2 f32
        eng.tensor_tensor(out=tb[:], in0=sl(0), in1=sl(6), op=Alu.add)
        eng.tensor_tensor(out=h3[:], in0=ta[:], in1=tb[:], op=Alu.add)
        hs[1].append(h1)
        hs[2].append(h2)
        hs[3].append(h3)

    # plane tiles RT[j] [128, 3, 2, 256] fp16 (BMP row-block j, all d, both b)
    rt_0 = work_pool.tile([128, 3, B, HP], F16, tag="rt_0")
    rt_1 = work_pool.tile([128, 3, B, HP], F16, tag="rt_1")
    rts = [rt_0, rt_1]
    for d in (1, 2, 3):
        area = float((2 * d + 1) ** 2)
        for j in range(2):
            ps = psumB_pool.tile([128, NB], F32, tag=f"bps{j}")
            sdt_t = sdt_lo if j == 0 else sdt_hi
            nc.tensor.matmul(out=ps[:], lhsT=sdt_t[:, d - 1, :],
                             rhs=hs[d][j][:].rearrange("r b w -> r (b w)"),
                             start=True, stop=True)
            rt = rts[j]
            nc.scalar.activation(rt[:, d - 1, :, PAD:PAD + W],
                                 ps[:].rearrange("r (b w) -> r b w", b=B),
                                 Act.Copy, scale=1.0 / area)
            nc.vector.tensor_copy(
                out=rt[:, d - 1, :, 0:PAD],
                in_=rt[:, d - 1, :, PAD:PAD + 1].to_broadcast((128, B, PAD)))
            nc.vector.tensor_copy(
                out=rt[:, d - 1, :, PAD + W:],
                in_=rt[:, d - 1, :, PAD + W - 1:PAD + W].to_broadcast((128, B, PAD)))

    # ---------------- Stage C: PE gather ----------------
    rt0_t, rt1_t = rts[0], rts[1]
    rt0_base = rt0_t[:].offset
    rt1_base = rt1_t[:].offset
    e2_base = e2t[:].offset
    assert isinstance(rt0_base, int) and isinstance(rt1_base, int)
    assert isinstance(e2_base, int)
    RT_AP = [[3 * B * HP, 128], [HP, B], [1, W]]   # [128, b, 224] window view
    MAX_RHS = 2 * 512 + 2 * PAD                    # 1056

    def rhs_ap(rt, base, off):
        return bass.AP(rt[:].tensor, base + off, [r[:] for r in RT_AP])

    def lhs_ap(off, m):
        return bass.AP(e2t[:].tensor, e2_base + off, [[2 * EW, 128], [1, m]])

    estage_pool = ctx.enter_context(tc.tile_pool(name="estage", bufs=3))

    n_grp = P_CORE // GRP
    for g in range(n_grp):
        st0 = stage_pool.tile([128, GRP, B, W], F16, tag="st0")
        st1 = stage_pool.tile([96, GRP, B, W], F16, tag="st1")
        for i in range(GRP):
            p = g * GRP + i
            _, pe_vals = nc.values_load_multi_w_load_instructions(
                tab_i[0:1, p, 0:2], engines=[EngT.PE],
                min_val=0, max_val=MAX_RHS, skip_runtime_bounds_check=True)
            or1, or2 = pe_vals
            _, dve_vals = nc.values_load_multi_w_load_instructions(
                tab_i[0:1, p, 2:6], engines=[EngT.DVE],
                min_val=0, max_val=2 * EW - 128, skip_runtime_bounds_check=True)
            sy1h, sy1v, sy2h, sy2v = dve_vals
            # stage the 4 per-pair E slices (dynamic free-dim offset on DVE)
            ea = estage_pool.tile([128, 128], F16, tag="ea")  # +E[128+sy1]
            eb = estage_pool.tile([128, 128], F16, tag="eb")  # +E[sy1]
            ec = estage_pool.tile([128, 128], F16, tag="ec")  # -E[128+sy2]
            ed = estage_pool.tile([128, 128], F16, tag="ed")  # -E[sy2]
            for et, off in ((ea, sy1h), (eb, sy1v), (ec, sy2h), (ed, sy2v)):
                nc.vector.tensor_copy(out=et[:], in_=lhs_ap(off, 128))
            ps0 = psum_pool.tile([128, NB], F32, tag="ps0")
            ps1 = psum_pool.tile([96, NB], F32, tag="ps1")
            nc.tensor.matmul(out=ps0[:], lhsT=ea[:],
                             rhs=rhs_ap(rt0_t, rt0_base, or1),
                             start=True, stop=False)
            nc.tensor.matmul(out=ps0[:], lhsT=eb[:],
                             rhs=rhs_ap(rt1_t, rt1_base, or1),
                             start=False, stop=False)
            nc.tensor.matmul(out=ps0[:], lhsT=ec[:],
                             rhs=rhs_ap(rt0_t, rt0_base, or2),
                             start=False, stop=False)
            nc.tensor.matmul(out=ps0[:], lhsT=ed[:],
                             rhs=rhs_ap(rt1_t, rt1_base, or2),
                             start=False, stop=True)
            nc.tensor.matmul(out=ps1[:], lhsT=ea[:, 0:96],
                             rhs=rhs_ap(rt1_t, rt1_base, or1),
                             start=True, stop=False)
            nc.tensor.matmul(out=ps1[:], lhsT=ec[:, 0:96],
                             rhs=rhs_ap(rt1_t, rt1_base, or2),
                             start=False, stop=True)
            nc.scalar.activation(st0[:, i, :, :],
                                 ps0[:].rearrange("r (b w) -> r b w", b=B),
                                 Act.Identity, bias=negthr[0:128, p:p + 1])
            nc.scalar.activation(st1[:, i, :, :],
                                 ps1[:].rearrange("r (b w) -> r b w", b=B),
                                 Act.Identity, bias=negthr[0:96, p:p + 1])
        nc.sync.dma_start(out=out0_ap[:, g * GRP:(g + 1) * GRP], in_=st0[:])
        nc.sync.dma_start(out=out1_ap[:, g * GRP:(g + 1) * GRP], in_=st1[:])

    ctx.close()


_COMPILED = {}


def _get_compiled():
    if "nc" not in _COMPILED:
        nc = bacc.Bacc("TRN2", target_bir_lowering=False, debug=False,
                       num_devices=N_CORES)
        build_device_program(nc)
        nc.compile()
        _COMPILED["nc"] = nc
    return _COMPILED["nc"]


def _ensure_ntff_hook():
    """The agent image's antenv lacks axon_hooks; shim it so trace=True can
    drive NTFF profiling via the boot module's ctypes hook (test-only path)."""
    import types

    try:
        from antenv.axon_hooks import get_axon_ntff_profile_hook  # noqa: F401
        return
    except ImportError:
        pass
    import antenv

    mod = types.ModuleType("antenv.axon_hooks")
    _hook = [None]
    mod.set_axon_ntff_profile_hook = lambda h: _hook.__setitem__(0, h)
    mod.get_axon_ntff_profile_hook = lambda: _hook[0]
    sys.modules["antenv.axon_hooks"] = mod
    antenv.axon_hooks = mod
    from trn_agent_boot.trn_boot import _ntff_profile_via_ctypes

    mod.set_axon_ntff_profile_hook(
        _ntff_profile_via_ctypes("/opt/axon/libaxon_pjrt.so"))


def run(inputs: dict, trace: bool = False):
    """Run on the 8 cores. Returns (full output [B,256,H,W], exec_time_ns|None)."""
    x = np.asarray(inputs["x"], dtype=np.float32).reshape(B, H, W)
    offset_x1 = np.asarray(inputs["offset_x1"], np.float32)
    offset_x2 = np.asarray(inputs["offset_x2"], np.float32)
    offset_y1 = np.asarray(inputs["offset_y1"], np.float32)
    offset_y2 = np.asarray(inputs["offset_y2"], np.float32)
    radii = np.asarray(inputs["radii"]).astype(np.int32)
    thresholds = np.asarray(inputs["thresholds"], np.float32)

    sdt = _band_matrices()
    e2 = _shift_identity()
    nc = _get_compiled()

    in_maps = []
    for c in range(N_CORES):
        sl = slice(c * P_CORE, (c + 1) * P_CORE)
        vecs = np.stack([offset_y1[sl], offset_x1[sl], offset_y2[sl],
                         offset_x2[sl], -thresholds[sl]]).astype(np.float32)
        in_maps.append({
            "x": x,
            "vecs": vecs,
            "radii": radii[sl].reshape(1, P_CORE),
            "sdt": sdt,
            "e2": e2,
        })

    if trace:
        _ensure_ntff_hook()
    res = run_bass_kernel_spmd(nc, in_maps, list(range(N_CORES)), trace=trace)
    # per-core out0 [128, P_CORE, B, W], out1 [96, P_CORE, B, W] fp16
    full = np.empty((B, P_TOTAL, H, W), np.float32)
    for c in range(N_CORES):
        o0 = res.results[c]["out0"].astype(np.float32)  # [128, 32, 2, 224]
        o1 = res.results[c]["out1"].astype(np.float32)  # [96, 32, 2, 224]
        sl = slice(c * P_CORE, (c + 1) * P_CORE)
        full[:, sl, :128, :] = o0.transpose(2, 1, 0, 3)
        full[:, sl, 128:, :] = o1.transpose(2, 1, 0, 3)
    return full, res.exec_time_ns


def kernel(x, offset_x1, offset_x2, offset_y1, offset_y2, radii, thresholds,
           max_radius):
    out, _ = run({
        "x": x, "offset_x1": offset_x1, "offset_x2": offset_x2,
        "offset_y1": offset_y1, "offset_y2": offset_y2,
        "radii": radii, "thresholds": thresholds, "max_radius": max_radius,
    })
    return out


if __name__ == "__main__":
    # smoke test with random data
    rng = np.random.default_rng(0)
    out = kernel(
        x=rng.standard_normal((B, 1, H, W), dtype=np.float32),
        offset_x1=rng.uniform(-16, 16, P_TOTAL).astype(np.float32),
        offset_x2=rng.uniform(-16, 16, P_TOTAL).astype(np.float32),
        offset_y1=rng.uniform(-16, 16, P_TOTAL).astype(np.float32),
        offset_y2=rng.uniform(-16, 16, P_TOTAL).astype(np.float32),
        radii=rng.integers(1, 4, P_TOTAL).astype(np.int32),
        thresholds=(rng.standard_normal(P_TOTAL) * 0.1).astype(np.float32),
        max_radius=3,
    )
    print("out", out.shape, out.dtype, float(np.abs(out).max()))


# revision 9
# speedup vs baseline: 1.8280x; 1.2132x over previous
"""BAD-descriptor kernel for Trainium2 (8 NeuronCores, SPMD over pairs).

Math: the reference gathers from an integral image at
  cy = clip(h + off_y, 0, H-1).astype(int) + r,  y0/y1 = cy -/+ rad(+1)
Because h is an integer grid, clip(h+off).astype(int) == clip(h + floor(off), 0, H-1),
so each box-mean term is the radius-d box-mean image sampled at a clamped
integer 2D shift.  With only 3 radii we precompute, per batch b and d in {1,2,3},
the box-mean image BM_d (edge-replicate semantics of the reference integral
image), padded by 16 with edge replication into BMP_d [256,256]:

  out[b,p] = BMP_{d_p}[b][sy1:sy1+224, sx1:sx1+224]
           - BMP_{d_p}[b][sy2:sy2+224, sx2:sx2+224] - thr_p,
  sy = floor(off_y)+16 in [0,32], sx likewise.

v3 (PE-gather, 112-row blocks): the 2D-shifted window read runs on the TENSOR
engine; the only HBM traffic is the input image and the fp16 output.

  out[m, n] = sum_k E[k, m] * P[k, sx + n]    E[k, m] = d(k == m + t)

where P is one of six 128-row BMP tiles at starts A = {0,16,32,112,128,144},
picked by sy: block0 (rows 0..111) uses a0 = 16*floor(sy/16), block1 (rows
112..223) uses a1 = a0 + 112, and both share t = sy mod 16 in [0,15].  The
tile index folds into the rhs free-dim dynamic offset (values_load regs); the
single per-window lhsT slice E[128+t : 240+t] is staged by one DVE copy
(dynamic free offset) because ldweights cannot take register offsets.  The
W1 - W2 subtraction is free via PSUM accumulation of +E / -E windows:
per pair 4 matmuls (K=128, M=112, N=448) -> ps0/ps1 [112,448]; ACT/DVE evict
with bias=-thr to fp16 staging; one output DMA per 8 pairs.  Tiles 0/4 are
written directly by stage-B evictions; tiles 1,2,3,5 are partition-shifted
SBUF->SBUF DMA copies.  Register loads are batched 8 pairs per TENSOR_LOAD
(~0.6us fixed cost each).
"""

import sys

sys.path.insert(0, "/opt/trn_rl_repo")

import numpy as np

import concourse.bass as bass
import concourse.bacc as bacc
import concourse.mybir as mybir
import concourse.tile as tile
from concourse.bass_utils import run_bass_kernel_spmd

B = 2
H = W = 224
P_TOTAL = 256
N_CORES = 8
P_CORE = P_TOTAL // N_CORES  # 32
PAD = 16
RMAX = 3
HP = H + 2 * PAD  # 256 padded image rows/cols
F32 = mybir.dt.float32
F16 = mybir.dt.float16
I32 = mybir.dt.int32

NB = B * W        # 448 matmul N (b, w)
HB = 112          # output row block height (M)
GRP = 8           # pairs per output DMA / per register-load batch
EW = 384          # identity block width (j dim) per sign
NT = 6            # plane row-tiles, starts {0,16,32,112,128,144}
TFREE = 3 * B * HP  # 1536 free elems per plane tile
D1 = 3 * TFREE      # block1 rhs offset delta (tiles 3..5 vs 0..2)


def _band_matrices() -> np.ndarray:
    """Vertical band matrices with the +-16 replicate pad baked in.

    sdt[0][r, d-1, m]: hs-tile0 row r (x rows 0..127) -> BMP row m
        (m in [0,128): h = max(m-16, 0)).
    sdt[1][k, d-1, m]: hs-tile1 row 96+k -> BMP row 128+m
        (h = min(112+m, 223)).
    entry = #{i in [-d,d] : clip(h+i, 0, H-1) == row}.
    """
    sdt = np.zeros((2, 128, 3, 128), np.float16)
    for d in (1, 2, 3):
        for m in range(128):
            h_lo = max(m - PAD, 0)
            h_hi = min(112 + m, H - 1)
            for i in range(-d, d + 1):
                r = min(max(h_lo + i, 0), H - 1)
                if r < 128:
                    sdt[0][r, d - 1, m] += 1.0
                r = min(max(h_hi + i, 0), H - 1)
                if 96 <= r:
                    sdt[1][r - 96, d - 1, m] += 1.0
    return sdt


def _shift_identity() -> np.ndarray:
    """e2 [128, 2*EW] fp16: e2[k, j] = d(k == j-128), e2[k, EW+j] = -d(k == j-128)."""
    e = np.zeros((128, 2 * EW), np.float16)
    for k in range(128):
        e[k, 128 + k] = 1.0
        e[k, EW + 128 + k] = -1.0
    return e


def build_device_program(nc: bacc.Bacc):
    x_ap = nc.dram_tensor("x", [B, H, W], F32, kind="ExternalInput").ap()
    # rows: offy1, offx1, offy2, offx2, NEGATED thr
    vecs_ap = nc.dram_tensor("vecs", [5, P_CORE], F32, kind="ExternalInput").ap()
    radii_ap = nc.dram_tensor("radii", [1, P_CORE], I32, kind="ExternalInput").ap()
    sdt_ap = nc.dram_tensor("sdt", [2, 128, 3, 128], F16, kind="ExternalInput").ap()
    e2_ap = nc.dram_tensor("e2", [128, 2 * EW], F16, kind="ExternalInput").ap()
    # fp16 outputs: block0 rows 0..111, block1 rows 112..223; [hpart, p, b, w]
    out0_ap = nc.dram_tensor("out0", [HB, P_CORE, B, W], F16,
                             kind="ExternalOutput").ap()
    out1_ap = nc.dram_tensor("out1", [HB, P_CORE, B, W], F16,
                             kind="ExternalOutput").ap()

    with tile.TileContext(nc) as tc:
        build_kernel(tc, out0_ap, out1_ap, x_ap, vecs_ap, radii_ap, sdt_ap, e2_ap)
    return nc


def build_kernel(tc, out0_ap, out1_ap, x_ap, vecs_ap, radii_ap, sdt_ap, e2_ap):
    nc = tc.nc
    EngT = mybir.EngineType
    Alu = mybir.AluOpType
    Act = mybir.ActivationFunctionType

    from contextlib import ExitStack
    ctx = ExitStack()
    const_pool = ctx.enter_context(tc.tile_pool(name="const", bufs=1))
    work_pool = ctx.enter_context(tc.tile_pool(name="work", bufs=1))
    psumB_pool = ctx.enter_context(tc.tile_pool(name="psumB", bufs=1, space="PSUM"))
    psum_pool = ctx.enter_context(tc.tile_pool(name="psum", bufs=3, space="PSUM"))
    stage_pool = ctx.enter_context(tc.tile_pool(name="stage", bufs=2))
    estage_pool = ctx.enter_context(tc.tile_pool(name="estage", bufs=3))

    # ---------------- Stage A: pair prep ----------------
    vt = const_pool.tile([1, 5, P_CORE], F32, tag="v_all")
    nc.scalar.dma_start(out=vt[:], in_=vecs_ap[:])
    vecs = {name: vt[0:1, i, :] for i, name in enumerate(
        ("offy1", "offx1", "offy2", "offx2", "negthr"))}
    radii_t = const_pool.tile([1, P_CORE], I32, tag="v_radii")
    nc.scalar.dma_start(out=radii_t[:], in_=radii_ap[:])

    radf = const_pool.tile([1, P_CORE], F32, tag="radf")
    nc.vector.tensor_copy(out=radf[:], in_=radii_t[:])
    nc.vector.tensor_scalar(out=radf[:], in0=radf[:], scalar1=1.0, scalar2=3.0,
                            op0=Alu.max, op1=Alu.min)
    # dbase = (d-1)*512
    dbase = const_pool.tile([1, P_CORE], F32, tag="dbase")
    nc.vector.tensor_scalar(out=dbase[:], in0=radf[:], scalar1=1.0, scalar2=512.0,
                            op0=Alu.subtract, op1=Alu.mult)

    def floor_to_base(off_t, name):
        """return [1,P_CORE] f32 tile with clip(floor(off),-16,16)+16 in [0,32]."""
        ti = const_pool.tile([1, P_CORE], I32, tag=f"fi_{name}")
        tf = const_pool.tile([1, P_CORE], F32, tag=f"ff_{name}")
        gt = const_pool.tile([1, P_CORE], F32, tag=f"gt_{name}")
        res = const_pool.tile([1, P_CORE], F32, tag=f"fl_{name}")
        nc.vector.tensor_copy(out=ti[:], in_=off_t[:])   # cast (round or trunc)
        nc.vector.tensor_copy(out=tf[:], in_=ti[:])      # back to f32, exact
        nc.vector.tensor_tensor(out=gt[:], in0=tf[:], in1=off_t[:], op=Alu.is_gt)
        nc.vector.tensor_tensor(out=res[:], in0=tf[:], in1=gt[:], op=Alu.subtract)
        nc.vector.tensor_scalar_add(out=res[:], in0=res[:], scalar1=float(PAD))
        nc.vector.tensor_scalar(out=res[:], in0=res[:], scalar1=0.0,
                                scalar2=float(2 * PAD), op0=Alu.max, op1=Alu.min)
        return res

    sy1 = floor_to_base(vecs["offy1"], "y1")
    sx1 = floor_to_base(vecs["offx1"], "x1")
    sy2 = floor_to_base(vecs["offy2"], "y2")
    sx2 = floor_to_base(vecs["offx2"], "x2")

    def split_sy(sy, name):
        """b = (sy>=16)+(sy>=32) in {0,1,2};  t = sy - 16*b in [0,15]."""
        b16 = const_pool.tile([1, P_CORE], F32, tag=f"b16_{name}")
        b32 = const_pool.tile([1, P_CORE], F32, tag=f"b32_{name}")
        bt = const_pool.tile([1, P_CORE], F32, tag=f"bt_{name}")
        tt = const_pool.tile([1, P_CORE], F32, tag=f"tt_{name}")
        nc.vector.tensor_scalar(out=b16[:], in0=sy[:], scalar1=16.0, scalar2=None,
                                op0=Alu.is_ge)
        nc.vector.tensor_scalar(out=b32[:], in0=sy[:], scalar1=32.0, scalar2=None,
                                op0=Alu.is_ge)
        nc.vector.tensor_tensor(out=bt[:], in0=b16[:], in1=b32[:], op=Alu.add)
        nc.vector.tensor_scalar(out=tt[:], in0=bt[:], scalar1=-16.0, scalar2=None,
                                op0=Alu.mult)
        nc.vector.tensor_tensor(out=tt[:], in0=tt[:], in1=sy[:], op=Alu.add)
        return bt, tt

    b1, t1 = split_sy(sy1, "1")
    b2, t2 = split_sy(sy2, "2")

    # register tables, one load per GRP pairs per engine:
    #   tab_pe [1, n_grp, GRP, 2]: off_w = b_w*TFREE + (d-1)*512 + sx_w
    #   tab_dve [1, n_grp, GRP, 2]: e1 = 128 + t1,  e2 = EW + 128 + t2
    n_grp = P_CORE // GRP
    tabf_pe = const_pool.tile([1, n_grp, GRP, 2], F32, tag="tabf_pe")
    tabf_dve = const_pool.tile([1, n_grp, GRP, 2], F32, tag="tabf_dve")
    tmp = const_pool.tile([1, P_CORE], F32, tag="tab_tmp")
    for w, (bw, sxw) in enumerate(((b1, sx1), (b2, sx2))):
        nc.vector.tensor_scalar(out=tmp[:], in0=bw[:], scalar1=float(TFREE),
                                scalar2=None, op0=Alu.mult)
        nc.vector.tensor_tensor(out=tmp[:], in0=tmp[:], in1=dbase[:], op=Alu.add)
        nc.vector.tensor_tensor(
            out=tabf_pe[0:1, :, :, w].rearrange("a g i -> a (g i)"),
            in0=tmp[:], in1=sxw[:], op=Alu.add)
    nc.vector.tensor_scalar_add(
        out=tabf_dve[0:1, :, :, 0].rearrange("a g i -> a (g i)"),
        in0=t1[:], scalar1=128.0)
    nc.vector.tensor_scalar_add(
        out=tabf_dve[0:1, :, :, 1].rearrange("a g i -> a (g i)"),
        in0=t2[:], scalar1=float(EW + 128))
    tab_pe = const_pool.tile([1, n_grp, GRP, 2], I32, tag="tab_pe")
    tab_dve = const_pool.tile([1, n_grp, GRP, 2], I32, tag="tab_dve")
    nc.vector.tensor_copy(out=tab_pe[:], in_=tabf_pe[:])
    nc.vector.tensor_copy(out=tab_dve[:], in_=tabf_dve[:])

    # negated thresholds broadcast to 128 partitions (eviction bias, fp32)
    negthr = const_pool.tile([128, P_CORE], F32, tag="negthr")
    nc.scalar.dma_start(out=negthr[:],
                        in_=vecs_ap[4:5, :].to_broadcast((128, P_CORE)))

    # ---------------- Stage B: box-mean plane tiles in SBUF (fp16) -------
    part_rows = ((0, 128), (96, 128))  # (row0, nrows) x-row tiles (overlapping)

    xhs = []
    for j, (r0, nr) in enumerate(part_rows):
        xt = work_pool.tile([nr, B, W + 2 * RMAX], F32, tag=f"xt_{j}")
        for b in range(B):
            eng = nc.sync if b == 0 else nc.scalar
            eng.dma_start(out=xt[:, b, RMAX:RMAX + W], in_=x_ap[b, r0:r0 + nr, :])
        xh = work_pool.tile([nr, B, W + 2 * RMAX], F16, tag=f"xh_{j}")
        nc.vector.tensor_copy(out=xh[:, :, RMAX:RMAX + W],
                              in_=xt[:, :, RMAX:RMAX + W])
        nc.vector.tensor_copy(
            out=xh[:, :, 0:RMAX],
            in_=xh[:, :, RMAX:RMAX + 1].to_broadcast((nr, B, RMAX)))
        nc.vector.tensor_copy(
            out=xh[:, :, RMAX + W:],
            in_=xh[:, :, RMAX + W - 1:RMAX + W].to_broadcast((nr, B, RMAX)))
        xhs.append(xh)

    sdt_lo = const_pool.tile([128, 3, 128], F16, tag="sdt_lo")
    sdt_hi = const_pool.tile([128, 3, 128], F16, tag="sdt_hi")
    nc.sync.dma_start(out=sdt_lo[:], in_=sdt_ap[0])
    nc.scalar.dma_start(out=sdt_hi[:], in_=sdt_ap[1])
    e2t = const_pool.tile([128, 2 * EW], F16, tag="e2t")
    nc.sync.dma_start(out=e2t[:], in_=e2_ap[:])

    # horizontal box sums hs[d][j]: [nr, B, W] fp16 (fp16 taps -> DVE 2x mode)
    hs = {1: [], 2: [], 3: []}
    for j, (r0, nr) in enumerate(part_rows):
        xh = xhs[j]
        eng = nc.vector
        h1 = work_pool.tile([nr, B, W], F16, tag=f"hs1_{j}")
        h2 = work_pool.tile([nr, B, W], F16, tag=f"hs2_{j}")
        h3 = work_pool.tile([nr, B, W], F16, tag=f"hs3_{j}")
        ta = work_pool.tile([nr, B, W], F16, tag=f"hta_{j}")
        sl = lambda c: xh[:, :, c:c + W]
        eng.tensor_tensor(out=ta[:], in0=sl(2), in1=sl(3), op=Alu.add)
        eng.tensor_tensor(out=h1[:], in0=ta[:], in1=sl(4), op=Alu.add)
        eng.tensor_tensor(out=ta[:], in0=sl(1), in1=sl(5), op=Alu.add)
        eng.tensor_tensor(out=h2[:], in0=h1[:], in1=ta[:], op=Alu.add)
        eng.tensor_tensor(out=ta[:], in0=sl(0), in1=sl(6), op=Alu.add)
        eng.tensor_tensor(out=h3[:], in0=h2[:], in1=ta[:], op=Alu.add)
        hs[1].append(h1)
        hs[2].append(h2)
        hs[3].append(h3)

    # plane tensor [128, NT, 3, B, HP] fp16; tile starts {0,16,32,112,128,144}.
    # Tiles 0 (BMP rows 0..127) and 4 (128..255) are evicted directly from
    # PSUM; tiles 1,2,3,5 are partition-shifted S->S DMA copies.
    pl = work_pool.tile([128, NT, 3, B, HP], F16, tag="pl")
    for d in (1, 2, 3):
        area = float((2 * d + 1) ** 2)
        for j, tix in ((0, 0), (1, 4)):
            ps = psumB_pool.tile([128, NB], F32, tag=f"bps{j}")
            sdt_t = sdt_lo if j == 0 else sdt_hi
            nc.tensor.matmul(out=ps[:], lhsT=sdt_t[:, d - 1, :],
                             rhs=hs[d][j][:].rearrange("r b w -> r (b w)"),
                             start=True, stop=True)
            nc.scalar.activation(pl[:, tix, d - 1, :, PAD:PAD + W],
                                 ps[:].rearrange("r (b w) -> r b w", b=B),
                                 Act.Copy, scale=1.0 / area)
            nc.vector.tensor_copy(
                out=pl[:, tix, d - 1, :, 0:PAD],
                in_=pl[:, tix, d - 1, :, PAD:PAD + 1].to_broadcast((128, B, PAD)))
            nc.vector.tensor_copy(
                out=pl[:, tix, d - 1, :, PAD + W:],
                in_=pl[:, tix, d - 1, :, PAD + W - 1:PAD + W].to_broadcast(
                    (128, B, PAD)))
        # S->S partition-shifted copies for this d (per-d for precise deps):
        # (dst_tile, src_tile, src_p0, n, dst_p0)
        for k, (dt_, st_, sp, n, dp) in enumerate((
                (1, 0, 16, 112, 0), (1, 4, 0, 16, 112),
                (2, 0, 32, 96, 0), (2, 4, 0, 32, 96),
                (3, 0, 112, 16, 0), (3, 4, 0, 112, 16),
                (5, 4, 16, 112, 0), (5, 4, 112, 16, 112))):
            eng = nc.sync if k % 2 == 0 else nc.scalar
            eng.dma_start(out=pl[dp:dp + n, dt_, d - 1, :, :],
                          in_=pl[sp:sp + n, st_, d - 1, :, :])

    # ---------------- Stage C: PE gather ----------------
    pl_base = pl[:].offset
    e2_base = e2t[:].offset
    assert isinstance(pl_base, int) and isinstance(e2_base, int)
    PL_AP = [[NT * TFREE, 128], [HP, B], [1, W]]   # [128, b, 224] window view
    MAX_RHS = 2 * TFREE + 2 * 512 + 2 * PAD

    def rhs_ap(off, extra):
        return bass.AP(pl[:].tensor, pl_base + extra + off,
                       [r[:] for r in PL_AP])

    def lhs_ap(off):
        return bass.AP(e2t[:].tensor, e2_base + off, [[2 * EW, 128], [1, HB]])

    for g in range(n_grp):
        _, pe_vals = nc.values_load_multi_w_load_instructions(
            tab_pe[0:1, g, :, :], engines=[EngT.PE],
            min_val=0, max_val=MAX_RHS, skip_runtime_bounds_check=True)
        _, dve_vals = nc.values_load_multi_w_load_instructions(
            tab_dve[0:1, g, :, :], engines=[EngT.DVE],
            min_val=0, max_val=EW + 128 + 16, skip_runtime_bounds_check=True)
        st0 = stage_pool.tile([HB, GRP, B, W], F16, tag="st0")
        st1 = stage_pool.tile([HB, GRP, B, W], F16, tag="st1")
        for i in range(GRP):
            p = g * GRP + i
            or1, or2 = pe_vals[2 * i], pe_vals[2 * i + 1]
            oe1, oe2 = dve_vals[2 * i], dve_vals[2 * i + 1]
            es1 = estage_pool.tile([128, HB], F16, tag="es1")
            es2 = estage_pool.tile([128, HB], F16, tag="es2")
            nc.vector.tensor_copy(out=es1[:], in_=lhs_ap(oe1))
            nc.vector.tensor_copy(out=es2[:], in_=lhs_ap(oe2))
            ps0 = psum_pool.tile([HB, NB], F32, tag="ps0")
            ps1 = psum_pool.tile([HB, NB], F32, tag="ps1")
            nc.tensor.matmul(out=ps0[:], lhsT=es1[:], rhs=rhs_ap(or1, 0),
                             start=True, stop=False)
            nc.tensor.matmul(out=ps1[:], lhsT=es1[:], rhs=rhs_ap(or1, D1),
                             start=True, stop=False)
            nc.tensor.matmul(out=ps0[:], lhsT=es2[:], rhs=rhs_ap(or2, 0),
                             start=False, stop=True)
            nc.tensor.matmul(out=ps1[:], lhsT=es2[:], rhs=rhs_ap(or2, D1),
                             start=False, stop=True)
            nc.scalar.activation(st0[:, i, :, :],
                                 ps0[:].rearrange("r (b w) -> r b w", b=B),
                                 Act.Identity, bias=negthr[0:HB, p:p + 1])
            if i % 2 == 0:
                nc.scalar.activation(st1[:, i, :, :],
                                     ps1[:].rearrange("r (b w) -> r b w", b=B),
                                     Act.Identity, bias=negthr[0:HB, p:p + 1])
            else:
                nc.vector.tensor_scalar_add(
                    out=st1[:, i, :, :],
                    in0=ps1[:].rearrange("r (b w) -> r b w", b=B),
                    scalar1=negthr[0:HB, p:p + 1])
        nc.sync.dma_start(out=out0_ap[:, g * GRP:(g + 1) * GRP], in_=st0[:])
        nc.sync.dma_start(out=out1_ap[:, g * GRP:(g + 1) * GRP], in_=st1[:])

    ctx.close()


_COMPILED = {}


def _get_compiled():
    if "nc" not in _COMPILED:
        nc = bacc.Bacc("TRN2", target_bir_lowering=False, debug=False,
                       num_devices=N_CORES)
        build_device_program(nc)
        nc.compile()
        _COMPILED["nc"] = nc
    return _COMPILED["nc"]


def _ensure_ntff_hook():
    """The agent image's antenv lacks axon_hooks; shim it so trace=True can
    drive NTFF profiling via the boot module's ctypes hook (test-only path)."""
    import types

    try:
        from antenv.axon_hooks import get_axon_ntff_profile_hook  # noqa: F401
        return
    except ImportError:
        pass
    import antenv

    mod = types.ModuleType("antenv.axon_hooks")
    _hook = [None]
    mod.set_axon_ntff_profile_hook = lambda h: _hook.__setitem__(0, h)
    mod.get_axon_ntff_profile_hook = lambda: _hook[0]
    sys.modules["antenv.axon_hooks"] = mod
    antenv.axon_hooks = mod
    from trn_agent_boot.trn_boot import _ntff_profile_via_ctypes

    mod.set_axon_ntff_profile_hook(
        _ntff_profile_via_ctypes("/opt/axon/libaxon_pjrt.so"))


def run(inputs: dict, trace: bool = False):
    """Run on the 8 cores. Returns (full output [B,256,H,W], exec_time_ns|None)."""
    x = np.asarray(inputs["x"], dtype=np.float32).reshape(B, H, W)
    offset_x1 = np.asarray(inputs["offset_x1"], np.float32)
    offset_x2 = np.asarray(inputs["offset_x2"], np.float32)
    offset_y1 = np.asarray(inputs["offset_y1"], np.float32)
    offset_y2 = np.asarray(inputs["offset_y2"], np.float32)
    radii = np.asarray(inputs["radii"]).astype(np.int32)
    thresholds = np.asarray(inputs["thresholds"], np.float32)

    sdt = _band_matrices()
    e2 = _shift_identity()
    nc = _get_compiled()

    in_maps = []
    for c in range(N_CORES):
        sl = slice(c * P_CORE, (c + 1) * P_CORE)
        vecs = np.stack([offset_y1[sl], offset_x1[sl], offset_y2[sl],
                         offset_x2[sl], -thresholds[sl]]).astype(np.float32)
        in_maps.append({
            "x": x,
            "vecs": vecs,
            "radii": radii[sl].reshape(1, P_CORE),
            "sdt": sdt,
            "e2": e2,
        })

    if trace:
        _ensure_ntff_hook()
    res = run_bass_kernel_spmd(nc, in_maps, list(range(N_CORES)), trace=trace)
    # per-core out0/out1 [112, P_CORE, B, W] fp16 (rows 0..111 / 112..223)
    full = np.empty((B, P_TOTAL, H, W), np.float32)
    for c in range(N_CORES):
        o0 = res.results[c]["out0"].astype(np.float32)
        o1 = res.results[c]["out1"].astype(np.float32)
        sl = slice(c * P_CORE, (c + 1) * P_CORE)
        full[:, sl, :HB, :] = o0.transpose(2, 1, 0, 3)
        full[:, sl, HB:, :] = o1.transpose(2, 1, 0, 3)
    return full, res.exec_time_ns


def kernel(x, offset_x1, offset_x2, offset_y1, offset_y2, radii, thresholds,
           max_radius):
    out, _ = run({
        "x": x, "offset_x1": offset_x1, "offset_x2": offset_x2,
        "offset_y1": offset_y1, "offset_y2": offset_y2,
        "radii": radii, "thresholds": thresholds, "max_radius": max_radius,
    })
    return out


if __name__ == "__main__":
    # smoke test with random data
    rng = np.random.default_rng(0)
    out = kernel(
        x=rng.standard_normal((B, 1, H, W), dtype=np.float32),
        offset_x1=rng.uniform(-16, 16, P_TOTAL).astype(np.float32),
        offset_x2=rng.uniform(-16, 16, P_TOTAL).astype(np.float32),
        offset_y1=rng.uniform(-16, 16, P_TOTAL).astype(np.float32),
        offset_y2=rng.uniform(-16, 16, P_TOTAL).astype(np.float32),
        radii=rng.integers(1, 4, P_TOTAL).astype(np.int32),
        thresholds=(rng.standard_normal(P_TOTAL) * 0.1).astype(np.float32),
        max_radius=3,
    )
    print("out", out.shape, out.dtype, float(np.abs(out).max()))


# revision 12
# speedup vs baseline: 1.9707x; 1.0781x over previous
"""BAD-descriptor kernel for Trainium2 (8 NeuronCores, SPMD over pairs).

Math: the reference gathers from an integral image at
  cy = clip(h + off_y, 0, H-1).astype(int) + r,  y0/y1 = cy -/+ rad(+1)
Because h is an integer grid, clip(h+off).astype(int) == clip(h + floor(off), 0, H-1),
so each box-mean term is the radius-d box-mean image sampled at a clamped
integer 2D shift.  With only 3 radii we precompute, per batch b and d in {1,2,3},
the box-mean image BM_d (edge-replicate semantics of the reference integral
image), padded by 16 with edge replication into BMP_d [256,256]:

  out[b,p] = BMP_{d_p}[b][sy1:sy1+224, sx1:sx1+224]
           - BMP_{d_p}[b][sy2:sy2+224, sx2:sx2+224] - thr_p,
  sy = floor(off_y)+16 in [0,32], sx likewise.

v4 (PE-gather, 112-row blocks, bf16): the 2D-shifted window read runs on the
TENSOR engine; the only HBM traffic is the input image and the bf16 output.

  out[m, n] = sum_k E[k, m] * P[k, sx + n]    E[k, m] = d(k == m + t)

where P is one of six 128-row BMP tiles at starts A = {0,16,32,112,128,144},
picked by sy: block0 (rows 0..111) uses a0 = 16*floor(sy/16), block1 (rows
112..223) uses a1 = a0 + 112, and both share t = sy mod 16.  The tile index
folds into the rhs free-dim dynamic offset (values_load regs, batched 8 pairs
per ~1us TENSOR_LOAD); the per-window lhsT slice E[128+t : 240+t] is staged
by one DVE copy (ldweights cannot take register offsets).  W1 - W2 is free
via PSUM accumulation of +E / -E windows: per pair 4 matmuls (K=128, M=112,
N=448) -> ps0/ps1 [112,448]; ACT/DVE evict with bias=-thr into bf16 staging;
output DMA per 4 pairs on alternating queues.  Plane tiles 0/4 are written
directly by stage-B evictions; tiles 1,2,3,5 are partition-shifted S->S DMA
copies.  Stage-A index arithmetic runs on the idle GpSimd queue; warm-up
matmuls keep the PE DVFS ramp alive between stage B and the gather stream.
"""

import sys

sys.path.insert(0, "/opt/trn_rl_repo")

import numpy as np
import ml_dtypes

import concourse.bass as bass
import concourse.bacc as bacc
import concourse.mybir as mybir
import concourse.tile as tile
from concourse.bass_utils import run_bass_kernel_spmd

BF16_NP = ml_dtypes.bfloat16

B = 2
H = W = 224
P_TOTAL = 256
N_CORES = 8
P_CORE = P_TOTAL // N_CORES  # 32
PAD = 16
RMAX = 3
HP = H + 2 * PAD  # 256 padded image rows/cols
F32 = mybir.dt.float32
BF16 = mybir.dt.bfloat16
I32 = mybir.dt.int32

NB = B * W        # 448 matmul N (b, w)
HB = 112          # output row block height (M)
LGRP = 8          # pairs per register-load batch
OGRP = 4          # pairs per output DMA
EW = 384          # identity block width (j dim) per sign
NT = 6            # plane row-tiles, starts {0,16,32,112,128,144}
TFREE = 3 * B * HP  # 1536 free elems per plane tile
D1 = 3 * TFREE      # block1 rhs offset delta (tiles 3..5 vs 0..2)
N_WARM = 14         # PE warm-up matmuls between stage B and stage C


def _band_matrices() -> np.ndarray:
    """Vertical band matrices with the +-16 replicate pad baked in.

    sdt[0][r, d-1, m]: hs-tile0 row r (x rows 0..127) -> BMP row m
        (m in [0,128): h = max(m-16, 0)).
    sdt[1][k, d-1, m]: hs-tile1 row 96+k -> BMP row 128+m
        (h = min(112+m, 223)).
    entry = #{i in [-d,d] : clip(h+i, 0, H-1) == row}.
    """
    sdt = np.zeros((2, 128, 3, 128), BF16_NP)
    for d in (1, 2, 3):
        for m in range(128):
            h_lo = max(m - PAD, 0)
            h_hi = min(112 + m, H - 1)
            for i in range(-d, d + 1):
                r = min(max(h_lo + i, 0), H - 1)
                if r < 128:
                    sdt[0][r, d - 1, m] += BF16_NP(1.0)
                r = min(max(h_hi + i, 0), H - 1)
                if 96 <= r:
                    sdt[1][r - 96, d - 1, m] += BF16_NP(1.0)
    return sdt


def _shift_identity() -> np.ndarray:
    """e2 [128, 2*EW]: e2[k, j] = d(k == j-128), e2[k, EW+j] = -d(k == j-128)."""
    e = np.zeros((128, 2 * EW), BF16_NP)
    for k in range(128):
        e[k, 128 + k] = 1.0
        e[k, EW + 128 + k] = -1.0
    return e


def build_device_program(nc: bacc.Bacc):
    x_ap = nc.dram_tensor("x", [B, H, W], F32, kind="ExternalInput").ap()
    # rows: offy1, offx1, offy2, offx2, NEGATED thr
    vecs_ap = nc.dram_tensor("vecs", [5, P_CORE], F32, kind="ExternalInput").ap()
    radii_ap = nc.dram_tensor("radii", [1, P_CORE], I32, kind="ExternalInput").ap()
    sdt_ap = nc.dram_tensor("sdt", [2, 128, 3, 128], BF16, kind="ExternalInput").ap()
    e2_ap = nc.dram_tensor("e2", [128, 2 * EW], BF16, kind="ExternalInput").ap()
    # bf16 outputs: block0 rows 0..111, block1 rows 112..223; [hpart, p, b, w]
    out0_ap = nc.dram_tensor("out0", [HB, P_CORE, B, W], BF16,
                             kind="ExternalOutput").ap()
    out1_ap = nc.dram_tensor("out1", [HB, P_CORE, B, W], BF16,
                             kind="ExternalOutput").ap()

    with tile.TileContext(nc) as tc:
        build_kernel(tc, out0_ap, out1_ap, x_ap, vecs_ap, radii_ap, sdt_ap, e2_ap)
    return nc


def build_kernel(tc, out0_ap, out1_ap, x_ap, vecs_ap, radii_ap, sdt_ap, e2_ap):
    nc = tc.nc
    EngT = mybir.EngineType
    Alu = mybir.AluOpType
    Act = mybir.ActivationFunctionType

    from contextlib import ExitStack
    ctx = ExitStack()
    const_pool = ctx.enter_context(tc.tile_pool(name="const", bufs=1))
    work_pool = ctx.enter_context(tc.tile_pool(name="work", bufs=1))
    psumB_pool = ctx.enter_context(tc.tile_pool(name="psumB", bufs=1, space="PSUM"))
    psum_pool = ctx.enter_context(tc.tile_pool(name="psum", bufs=3, space="PSUM"))
    stage_pool = ctx.enter_context(tc.tile_pool(name="stage", bufs=2))
    estage_pool = ctx.enter_context(tc.tile_pool(name="estage", bufs=9))

    # ---------------- input DMAs ----------------
    vt = const_pool.tile([1, 5, P_CORE], F32, tag="v_all")
    nc.scalar.dma_start(out=vt[:], in_=vecs_ap[:])
    radii_t = const_pool.tile([1, P_CORE], I32, tag="v_radii")
    nc.scalar.dma_start(out=radii_t[:], in_=radii_ap[:])
    part_rows = ((0, 128), (96, 128))  # (row0, nrows) x-row tiles (overlapping)
    xts = []
    for j, (r0, nr) in enumerate(part_rows):
        xt = work_pool.tile([nr, B, W + 2 * RMAX], F32, tag=f"xt_{j}")
        for b in range(B):
            eng = nc.sync if b == 0 else nc.scalar
            eng.dma_start(out=xt[:, b, RMAX:RMAX + W], in_=x_ap[b, r0:r0 + nr, :])
        xts.append(xt)
    sdt_lo = const_pool.tile([128, 3, 128], BF16, tag="sdt_lo")
    sdt_hi = const_pool.tile([128, 3, 128], BF16, tag="sdt_hi")
    nc.sync.dma_start(out=sdt_lo[:], in_=sdt_ap[0])
    nc.scalar.dma_start(out=sdt_hi[:], in_=sdt_ap[1])
    e2t = const_pool.tile([128, 2 * EW], BF16, tag="e2t")
    nc.sync.dma_start(out=e2t[:], in_=e2_ap[:])
    negthr = const_pool.tile([128, P_CORE], F32, tag="negthr")
    nc.scalar.dma_start(out=negthr[:],
                        in_=vecs_ap[4:5, :].to_broadcast((128, P_CORE)))

    # ---------------- DVE: x cast + pads + horizontal taps ----------------
    xhs = []
    for j, (r0, nr) in enumerate(part_rows):
        xt = xts[j]
        xh = work_pool.tile([nr, B, W + 2 * RMAX], BF16, tag=f"xh_{j}")
        nc.vector.tensor_copy(out=xh[:, :, RMAX:RMAX + W],
                              in_=xt[:, :, RMAX:RMAX + W])
        nc.vector.tensor_copy(
            out=xh[:, :, 0:RMAX],
            in_=xh[:, :, RMAX:RMAX + 1].to_broadcast((nr, B, RMAX)))
        nc.vector.tensor_copy(
            out=xh[:, :, RMAX + W:],
            in_=xh[:, :, RMAX + W - 1:RMAX + W].to_broadcast((nr, B, RMAX)))
        xhs.append(xh)

    hs = {1: [], 2: [], 3: []}
    for j, (r0, nr) in enumerate(part_rows):
        xh = xhs[j]
        eng = nc.vector
        h1 = work_pool.tile([nr, B, W], BF16, tag=f"hs1_{j}")
        h2 = work_pool.tile([nr, B, W], BF16, tag=f"hs2_{j}")
        h3 = work_pool.tile([nr, B, W], BF16, tag=f"hs3_{j}")
        ta = work_pool.tile([nr, B, W], BF16, tag=f"hta_{j}")
        sl = lambda c: xh[:, :, c:c + W]
        eng.tensor_tensor(out=ta[:], in0=sl(2), in1=sl(3), op=Alu.add)
        eng.tensor_tensor(out=h1[:], in0=ta[:], in1=sl(4), op=Alu.add)
        eng.tensor_tensor(out=ta[:], in0=sl(1), in1=sl(5), op=Alu.add)
        eng.tensor_tensor(out=h2[:], in0=h1[:], in1=ta[:], op=Alu.add)
        eng.tensor_tensor(out=ta[:], in0=sl(0), in1=sl(6), op=Alu.add)
        eng.tensor_tensor(out=h3[:], in0=h2[:], in1=ta[:], op=Alu.add)
        hs[1].append(h1)
        hs[2].append(h2)
        hs[3].append(h3)

    # ---------------- Stage A (DVE, after taps): pair index prep ----------------
    # floor/clip all four offset rows at once: sfl [1, 4, P_CORE]
    #   rows: sy1, sx1, sy2, sx2 (vecs rows 0..3 are offy1, offx1, offy2, offx2)
    off4 = vt[0:1, 0:4, :]
    fi4 = const_pool.tile([1, 4, P_CORE], I32, tag="fi4")
    ff4 = const_pool.tile([1, 4, P_CORE], F32, tag="ff4")
    gt4 = const_pool.tile([1, 4, P_CORE], F32, tag="gt4")
    sfl = const_pool.tile([1, 4, P_CORE], F32, tag="sfl")
    nc.vector.tensor_copy(out=fi4[:], in_=off4)     # cast (round or trunc)
    nc.vector.tensor_copy(out=ff4[:], in_=fi4[:])   # back to f32, exact
    nc.vector.tensor_tensor(out=gt4[:], in0=ff4[:], in1=off4, op=Alu.is_gt)
    nc.vector.tensor_tensor(out=sfl[:], in0=ff4[:], in1=gt4[:], op=Alu.subtract)
    nc.vector.tensor_scalar_add(out=sfl[:], in0=sfl[:], scalar1=float(PAD))
    nc.vector.tensor_scalar(out=sfl[:], in0=sfl[:], scalar1=0.0,
                     scalar2=float(2 * PAD), op0=Alu.max, op1=Alu.min)
    sy1, sx1 = sfl[0:1, 0, :], sfl[0:1, 1, :]
    sy2, sx2 = sfl[0:1, 2, :], sfl[0:1, 3, :]
    syb = sfl[0:1, 0:3:2, :]  # both sy rows, stride 2

    # b = (sy>=16)+(sy>=32); t = sy - 16*b  (for both sy rows at once)
    bb = const_pool.tile([1, 2, P_CORE], F32, tag="bb")
    b32 = const_pool.tile([1, 2, P_CORE], F32, tag="b32")
    tt = const_pool.tile([1, 2, P_CORE], F32, tag="tt")
    nc.vector.tensor_scalar(out=bb[:], in0=syb, scalar1=16.0, scalar2=None, op0=Alu.is_ge)
    nc.vector.tensor_scalar(out=b32[:], in0=syb, scalar1=32.0, scalar2=None, op0=Alu.is_ge)
    nc.vector.tensor_tensor(out=bb[:], in0=bb[:], in1=b32[:], op=Alu.add)
    nc.vector.tensor_scalar(out=tt[:], in0=bb[:], scalar1=-16.0, scalar2=None, op0=Alu.mult)
    nc.vector.tensor_tensor(out=tt[:], in0=tt[:], in1=syb, op=Alu.add)

    radf = const_pool.tile([1, P_CORE], F32, tag="radf")
    nc.vector.tensor_copy(out=radf[:], in_=radii_t[:])
    nc.vector.tensor_scalar(out=radf[:], in0=radf[:], scalar1=1.0, scalar2=3.0,
                     op0=Alu.max, op1=Alu.min)
    dbase = const_pool.tile([1, P_CORE], F32, tag="dbase")
    nc.vector.tensor_scalar(out=dbase[:], in0=radf[:], scalar1=1.0, scalar2=512.0,
                     op0=Alu.subtract, op1=Alu.mult)

    # tab_pe [1, n_lgrp, LGRP, 2]: off_w = b_w*TFREE + (d-1)*512 + sx_w
    # tab_dve [1, n_lgrp, LGRP, 2]: e1 = 128 + t1,  e2 = EW + 128 + t2
    n_lgrp = P_CORE // LGRP
    tabf_pe = const_pool.tile([1, n_lgrp, LGRP, 2], F32, tag="tabf_pe")
    tabf_dve = const_pool.tile([1, n_lgrp, LGRP, 2], F32, tag="tabf_dve")
    tmp = const_pool.tile([1, P_CORE], F32, tag="tab_tmp")
    for w, sxw in enumerate((sx1, sx2)):
        nc.vector.tensor_scalar(out=tmp[:], in0=bb[0:1, w, :], scalar1=float(TFREE),
                         scalar2=None, op0=Alu.mult)
        nc.vector.tensor_tensor(out=tmp[:], in0=tmp[:], in1=dbase[:], op=Alu.add)
        nc.vector.tensor_tensor(
            out=tabf_pe[0:1, :, :, w].rearrange("a g i -> a (g i)"),
            in0=tmp[:], in1=sxw, op=Alu.add)
    nc.vector.tensor_scalar_add(
        out=tabf_dve[0:1, :, :, 0].rearrange("a g i -> a (g i)"),
        in0=tt[0:1, 0, :], scalar1=128.0)
    nc.vector.tensor_scalar_add(
        out=tabf_dve[0:1, :, :, 1].rearrange("a g i -> a (g i)"),
        in0=tt[0:1, 1, :], scalar1=float(EW + 128))
    tab_pe = const_pool.tile([1, n_lgrp, LGRP, 2], I32, tag="tab_pe")
    tab_dve = const_pool.tile([1, n_lgrp, LGRP, 2], I32, tag="tab_dve")
    nc.vector.tensor_copy(out=tab_pe[:], in_=tabf_pe[:])
    nc.vector.tensor_copy(out=tab_dve[:], in_=tabf_dve[:])

    # ---------------- Stage B: plane tiles (PE matmul + ACT evict) -------
    # plane tensor [128, NT, 3, B, HP] bf16; tile starts {0,16,32,112,128,144}.
    pl = work_pool.tile([128, NT, 3, B, HP], BF16, tag="pl")
    for d in (1, 2, 3):
        area = float((2 * d + 1) ** 2)
        for j, tix in ((0, 0), (1, 4)):
            ps = psumB_pool.tile([128, NB], F32, tag=f"bps{j}")
            sdt_t = sdt_lo if j == 0 else sdt_hi
            nc.tensor.matmul(out=ps[:], lhsT=sdt_t[:, d - 1, :],
                             rhs=hs[d][j][:].rearrange("r b w -> r (b w)"),
                             start=True, stop=True)
            nc.scalar.activation(pl[:, tix, d - 1, :, PAD:PAD + W],
                                 ps[:].rearrange("r (b w) -> r b w", b=B),
                                 Act.Copy, scale=1.0 / area)
            nc.vector.tensor_copy(
                out=pl[:, tix, d - 1, :, 0:PAD],
                in_=pl[:, tix, d - 1, :, PAD:PAD + 1].to_broadcast((128, B, PAD)))
            nc.vector.tensor_copy(
                out=pl[:, tix, d - 1, :, PAD + W:],
                in_=pl[:, tix, d - 1, :, PAD + W - 1:PAD + W].to_broadcast(
                    (128, B, PAD)))

    # warm-up matmuls: keep the PE busy (DVFS ramp) while the S->S plane
    # copies run; results are discarded.
    for wmm in range(N_WARM):
        wps = psumB_pool.tile([128, NB], F32, tag=f"bps{wmm % 2}")
        nc.tensor.matmul(out=wps[:], lhsT=sdt_lo[:, 0, :], rhs=e2t[:, 0:NB],
                         start=True, stop=True)

    # S->S partition-shifted copies (all d at once):
    # (dst_tile, src_tile, src_p0, n, dst_p0); tile5 rows 112.. are filler
    # (finite values only, never selected by E).
    for k, (dt_, st_, sp, n, dp) in enumerate((
            (1, 0, 16, 112, 0), (1, 4, 0, 16, 112),
            (2, 0, 32, 96, 0), (2, 4, 0, 32, 96),
            (3, 0, 112, 16, 0), (3, 4, 0, 112, 16),
            (5, 4, 16, 112, 0), (5, 4, 112, 16, 112))):
        eng = nc.sync if k % 2 == 0 else nc.scalar
        eng.dma_start(out=pl[dp:dp + n, dt_, :, :, :],
                      in_=pl[sp:sp + n, st_, :, :, :])

    # ---------------- Stage C: PE gather ----------------
    pl_base = pl[:].offset
    e2_base = e2t[:].offset
    assert isinstance(pl_base, int) and isinstance(e2_base, int)
    PL_AP = [[NT * TFREE, 128], [HP, B], [1, W]]   # [128, b, 224] window view
    MAX_RHS = 2 * TFREE + 2 * 512 + 2 * PAD

    def rhs_ap(off, extra):
        return bass.AP(pl[:].tensor, pl_base + extra + off,
                       [r[:] for r in PL_AP])

    def lhs_ap(off):
        return bass.AP(e2t[:].tensor, e2_base + off, [[2 * EW, 128], [1, HB]])

    n_ogrp = P_CORE // OGRP
    sts = {}
    for g in range(n_lgrp):
        _, pe_vals = nc.values_load_multi_w_load_instructions(
            tab_pe[0:1, g, :, :], engines=[EngT.PE],
            min_val=0, max_val=MAX_RHS, skip_runtime_bounds_check=True)
        _, dve_vals = nc.values_load_multi_w_load_instructions(
            tab_dve[0:1, g, :, :], engines=[EngT.DVE],
            min_val=0, max_val=EW + 128 + 16, skip_runtime_bounds_check=True)
        # stage all LGRP pairs' E slices up front so the PE never waits
        ess = []
        for i in range(LGRP):
            es1 = estage_pool.tile([128, HB], BF16, tag="es1")
            es2 = estage_pool.tile([128, HB], BF16, tag="es2")
            nc.vector.tensor_copy(out=es1[:], in_=lhs_ap(dve_vals[2 * i]))
            nc.vector.tensor_copy(out=es2[:], in_=lhs_ap(dve_vals[2 * i + 1]))
            ess.append((es1, es2))
        for i in range(LGRP):
            p = g * LGRP + i
            og, oi = p // OGRP, p % OGRP
            if oi == 0:
                st0 = stage_pool.tile([HB, OGRP, B, W], BF16, tag="st0")
                st1 = stage_pool.tile([HB, OGRP, B, W], BF16, tag="st1")
                sts[og] = (st0, st1)
            st0, st1 = sts[og]
            or1, or2 = pe_vals[2 * i], pe_vals[2 * i + 1]
            es1, es2 = ess[i]
            ps0 = psum_pool.tile([HB, NB], F32, tag="ps0")
            ps1 = psum_pool.tile([HB, NB], F32, tag="ps1")
            nc.tensor.matmul(out=ps0[:], lhsT=es1[:], rhs=rhs_ap(or1, 0),
                             start=True, stop=False)
            nc.tensor.matmul(out=ps1[:], lhsT=es1[:], rhs=rhs_ap(or1, D1),
                             start=True, stop=False)
            nc.tensor.matmul(out=ps0[:], lhsT=es2[:], rhs=rhs_ap(or2, 0),
                             start=False, stop=True)
            nc.tensor.matmul(out=ps1[:], lhsT=es2[:], rhs=rhs_ap(or2, D1),
                             start=False, stop=True)
            nc.scalar.activation(st0[:, oi, :, :],
                                 ps0[:].rearrange("r (b w) -> r b w", b=B),
                                 Act.Identity, bias=negthr[0:HB, p:p + 1])
            if i % 4 == 3:
                nc.scalar.activation(st1[:, oi, :, :],
                                     ps1[:].rearrange("r (b w) -> r b w", b=B),
                                     Act.Identity, bias=negthr[0:HB, p:p + 1])
            else:
                nc.vector.tensor_scalar_add(
                    out=st1[:, oi, :, :],
                    in0=ps1[:].rearrange("r (b w) -> r b w", b=B),
                    scalar1=negthr[0:HB, p:p + 1])
            if oi == OGRP - 1:
                e0 = nc.sync if og % 2 == 0 else nc.scalar
                e1 = nc.scalar if og % 2 == 0 else nc.sync
                e0.dma_start(out=out0_ap[:, og * OGRP:(og + 1) * OGRP],
                             in_=st0[:])
                e1.dma_start(out=out1_ap[:, og * OGRP:(og + 1) * OGRP],
                             in_=st1[:])

    ctx.close()


_COMPILED = {}


def _get_compiled():
    if "nc" not in _COMPILED:
        nc = bacc.Bacc("TRN2", target_bir_lowering=False, debug=False,
                       num_devices=N_CORES)
        build_device_program(nc)
        nc.compile()
        _COMPILED["nc"] = nc
    return _COMPILED["nc"]


def _ensure_ntff_hook():
    """The agent image's antenv lacks axon_hooks; shim it so trace=True can
    drive NTFF profiling via the boot module's ctypes hook (test-only path)."""
    import types

    try:
        from antenv.axon_hooks import get_axon_ntff_profile_hook  # noqa: F401
        return
    except ImportError:
        pass
    import antenv

    mod = types.ModuleType("antenv.axon_hooks")
    _hook = [None]
    mod.set_axon_ntff_profile_hook = lambda h: _hook.__setitem__(0, h)
    mod.get_axon_ntff_profile_hook = lambda: _hook[0]
    sys.modules["antenv.axon_hooks"] = mod
    antenv.axon_hooks = mod
    from trn_agent_boot.trn_boot import _ntff_profile_via_ctypes

    mod.set_axon_ntff_profile_hook(
        _ntff_profile_via_ctypes("/opt/axon/libaxon_pjrt.so"))


def run(inputs: dict, trace: bool = False):
    """Run on the 8 cores. Returns (full output [B,256,H,W], exec_time_ns|None)."""
    x = np.asarray(inputs["x"], dtype=np.float32).reshape(B, H, W)
    offset_x1 = np.asarray(inputs["offset_x1"], np.float32)
    offset_x2 = np.asarray(inputs["offset_x2"], np.float32)
    offset_y1 = np.asarray(inputs["offset_y1"], np.float32)
    offset_y2 = np.asarray(inputs["offset_y2"], np.float32)
    radii = np.asarray(inputs["radii"]).astype(np.int32)
    thresholds = np.asarray(inputs["thresholds"], np.float32)

    sdt = _band_matrices()
    e2 = _shift_identity()
    nc = _get_compiled()

    in_maps = []
    for c in range(N_CORES):
        sl = slice(c * P_CORE, (c + 1) * P_CORE)
        vecs = np.stack([offset_y1[sl], offset_x1[sl], offset_y2[sl],
                         offset_x2[sl], -thresholds[sl]]).astype(np.float32)
        in_maps.append({
            "x": x,
            "vecs": vecs,
            "radii": radii[sl].reshape(1, P_CORE),
            "sdt": sdt,
            "e2": e2,
        })

    if trace:
        _ensure_ntff_hook()
    res = run_bass_kernel_spmd(nc, in_maps, list(range(N_CORES)), trace=trace)
    # per-core out0/out1 [112, P_CORE, B, W] bf16 (rows 0..111 / 112..223)
    full = np.empty((B, P_TOTAL, H, W), np.float32)
    for c in range(N_CORES):
        o0 = np.asarray(res.results[c]["out0"]).astype(np.float32)
        o1 = np.asarray(res.results[c]["out1"]).astype(np.float32)
        sl = slice(c * P_CORE, (c + 1) * P_CORE)
        full[:, sl, :HB, :] = o0.transpose(2, 1, 0, 3)
        full[:, sl, HB:, :] = o1.transpose(2, 1, 0, 3)
    return full, res.exec_time_ns


def kernel(x, offset_x1, offset_x2, offset_y1, offset_y2, radii, thresholds,
           max_radius):
    out, _ = run({
        "x": x, "offset_x1": offset_x1, "offset_x2": offset_x2,
        "offset_y1": offset_y1, "offset_y2": offset_y2,
        "radii": radii, "thresholds": thresholds, "max_radius": max_radius,
    })
    return out


if __name__ == "__main__":
    # smoke test with random data
    rng = np.random.default_rng(0)
    out = kernel(
        x=rng.standard_normal((B, 1, H, W), dtype=np.float32),
        offset_x1=rng.uniform(-16, 16, P_TOTAL).astype(np.float32),
        offset_x2=rng.uniform(-16, 16, P_TOTAL).astype(np.float32),
        offset_y1=rng.uniform(-16, 16, P_TOTAL).astype(np.float32),
        offset_y2=rng.uniform(-16, 16, P_TOTAL).astype(np.float32),
        radii=rng.integers(1, 4, P_TOTAL).astype(np.int32),
        thresholds=(rng.standard_normal(P_TOTAL) * 0.1).astype(np.float32),
        max_radius=3,
    )
    print("out", out.shape, out.dtype, float(np.abs(out).max()))


# revision 13
# speedup vs baseline: 2.1908x; 1.1117x over previous
"""BAD-descriptor kernel for Trainium2 (8 NeuronCores, SPMD over pairs).

Math: the reference gathers from an integral image at
  cy = clip(h + off_y, 0, H-1).astype(int) + r,  y0/y1 = cy -/+ rad(+1)
Because h is an integer grid, clip(h+off).astype(int) == clip(h + floor(off), 0, H-1),
so each box-mean term is the radius-d box-mean image sampled at a clamped
integer 2D shift.  With only 3 radii we precompute, per batch b and d in {1,2,3},
the box-mean image BM_d (edge-replicate semantics of the reference integral
image), padded by 16 with edge replication into BMP_d [256,256]:

  out[b,p] = BMP_{d_p}[b][sy1:sy1+224, sx1:sx1+224]
           - BMP_{d_p}[b][sy2:sy2+224, sx2:sx2+224] - thr_p,
  sy = floor(off_y)+16 in [0,32], sx likewise.

v4 (PE-gather, 112-row blocks, bf16): the 2D-shifted window read runs on the
TENSOR engine; the only HBM traffic is the input image and the bf16 output.

  out[m, n] = sum_k E[k, m] * P[k, sx + n]    E[k, m] = d(k == m + t)

where P is one of six 128-row BMP tiles at starts A = {0,16,32,112,128,144},
picked by sy: block0 (rows 0..111) uses a0 = 16*floor(sy/16), block1 (rows
112..223) uses a1 = a0 + 112, and both share t = sy mod 16.  The tile index
folds into the rhs free-dim dynamic offset (values_load regs, batched 8 pairs
per ~1us TENSOR_LOAD); the per-window lhsT slice E[128+t : 240+t] is staged
by one DVE copy (ldweights cannot take register offsets).  W1 - W2 is free
via PSUM accumulation of +E / -E windows: per pair 4 matmuls (K=128, M=112,
N=448) -> ps0/ps1 [112,448]; ACT/DVE evict with bias=-thr into bf16 staging;
output DMA per 4 pairs on alternating queues.  Plane tiles 0/4 are written
directly by stage-B evictions; tiles 1,2,3,5 are partition-shifted S->S DMA
copies.  Stage-A index arithmetic runs on the idle GpSimd queue; warm-up
matmuls keep the PE DVFS ramp alive between stage B and the gather stream.
"""

import sys

sys.path.insert(0, "/opt/trn_rl_repo")

import numpy as np
import ml_dtypes

import concourse.bass as bass
import concourse.bacc as bacc
import concourse.mybir as mybir
import concourse.tile as tile
from concourse.bass_utils import run_bass_kernel_spmd

BF16_NP = ml_dtypes.bfloat16

B = 2
H = W = 224
P_TOTAL = 256
N_CORES = 8
P_CORE = P_TOTAL // N_CORES  # 32
PAD = 16
RMAX = 3
HP = H + 2 * PAD  # 256 padded image rows/cols
F32 = mybir.dt.float32
BF16 = mybir.dt.bfloat16
I32 = mybir.dt.int32

NB = B * W        # 448 matmul N (b, w)
HB = 112          # output row block height (M)
LGRP = 8          # pairs per register-load batch
OGRP = 4          # pairs per output DMA
EW = 384          # identity block width (j dim) per sign
NT = 6            # plane row-tiles, starts {0,16,32,112,128,144}
TFREE = 3 * B * HP  # 1536 free elems per plane tile
D1 = 3 * TFREE      # block1 rhs offset delta (tiles 3..5 vs 0..2)
N_WARM = 18         # PE warm-up matmuls between stage B and stage C


def _band_matrices() -> np.ndarray:
    """Vertical band matrices with the +-16 replicate pad baked in.

    sdt[0][r, d-1, m]: hs-tile0 row r (x rows 0..127) -> BMP row m
        (m in [0,128): h = max(m-16, 0)).
    sdt[1][k, d-1, m]: hs-tile1 row 96+k -> BMP row 128+m
        (h = min(112+m, 223)).
    entry = #{i in [-d,d] : clip(h+i, 0, H-1) == row}.
    """
    sdt = np.zeros((2, 128, 3, 128), BF16_NP)
    for d in (1, 2, 3):
        for m in range(128):
            h_lo = max(m - PAD, 0)
            h_hi = min(112 + m, H - 1)
            for i in range(-d, d + 1):
                r = min(max(h_lo + i, 0), H - 1)
                if r < 128:
                    sdt[0][r, d - 1, m] += BF16_NP(1.0)
                r = min(max(h_hi + i, 0), H - 1)
                if 96 <= r:
                    sdt[1][r - 96, d - 1, m] += BF16_NP(1.0)
    return sdt


def _shift_identity() -> np.ndarray:
    """e2 [128, 2*EW]: e2[k, j] = d(k == j-128), e2[k, EW+j] = -d(k == j-128)."""
    e = np.zeros((128, 2 * EW), BF16_NP)
    for k in range(128):
        e[k, 128 + k] = 1.0
        e[k, EW + 128 + k] = -1.0
    return e


def build_device_program(nc: bacc.Bacc):
    x_ap = nc.dram_tensor("x", [B, H, W], F32, kind="ExternalInput").ap()
    # rows: offy1, offx1, offy2, offx2, NEGATED thr
    vecs_ap = nc.dram_tensor("vecs", [5, P_CORE], F32, kind="ExternalInput").ap()
    radii_ap = nc.dram_tensor("radii", [1, P_CORE], I32, kind="ExternalInput").ap()
    sdt_ap = nc.dram_tensor("sdt", [2, 128, 3, 128], BF16, kind="ExternalInput").ap()
    e2_ap = nc.dram_tensor("e2", [128, 2 * EW], BF16, kind="ExternalInput").ap()
    # bf16 outputs: block0 rows 0..111, block1 rows 112..223; [hpart, p, b, w]
    out0_ap = nc.dram_tensor("out0", [HB, P_CORE, B, W], BF16,
                             kind="ExternalOutput").ap()
    out1_ap = nc.dram_tensor("out1", [HB, P_CORE, B, W], BF16,
                             kind="ExternalOutput").ap()

    with tile.TileContext(nc) as tc:
        build_kernel(tc, out0_ap, out1_ap, x_ap, vecs_ap, radii_ap, sdt_ap, e2_ap)
    return nc


def build_kernel(tc, out0_ap, out1_ap, x_ap, vecs_ap, radii_ap, sdt_ap, e2_ap):
    nc = tc.nc
    EngT = mybir.EngineType
    Alu = mybir.AluOpType
    Act = mybir.ActivationFunctionType

    from contextlib import ExitStack
    ctx = ExitStack()
    const_pool = ctx.enter_context(tc.tile_pool(name="const", bufs=1))
    work_pool = ctx.enter_context(tc.tile_pool(name="work", bufs=1))
    psumB_pool = ctx.enter_context(tc.tile_pool(name="psumB", bufs=1, space="PSUM"))
    psum_pool = ctx.enter_context(tc.tile_pool(name="psum", bufs=3, space="PSUM"))
    stage_pool = ctx.enter_context(tc.tile_pool(name="stage", bufs=2))
    estage_pool = ctx.enter_context(tc.tile_pool(name="estage", bufs=9))

    # ---------------- input DMAs (x first: it gates the whole chain) -----
    part_rows = ((0, 128), (96, 128))  # (row0, nrows) x-row tiles (overlapping)
    xts = []
    for j, (r0, nr) in enumerate(part_rows):
        xt = work_pool.tile([nr, B, W + 2 * RMAX], F32, tag=f"xt_{j}")
        for b in range(B):
            eng = nc.sync if b == 0 else nc.scalar
            eng.dma_start(out=xt[:, b, RMAX:RMAX + W], in_=x_ap[b, r0:r0 + nr, :])
        xts.append(xt)
    sdt_lo = const_pool.tile([128, 3, 128], BF16, tag="sdt_lo")
    sdt_hi = const_pool.tile([128, 3, 128], BF16, tag="sdt_hi")
    nc.sync.dma_start(out=sdt_lo[:], in_=sdt_ap[0])
    nc.scalar.dma_start(out=sdt_hi[:], in_=sdt_ap[1])
    vt = const_pool.tile([1, 5, P_CORE], F32, tag="v_all")
    nc.scalar.dma_start(out=vt[:], in_=vecs_ap[:])
    radii_t = const_pool.tile([1, P_CORE], I32, tag="v_radii")
    nc.scalar.dma_start(out=radii_t[:], in_=radii_ap[:])
    e2t = const_pool.tile([128, 2 * EW], BF16, tag="e2t")
    nc.sync.dma_start(out=e2t[:], in_=e2_ap[:])
    negthr = const_pool.tile([128, P_CORE], F32, tag="negthr")
    nc.scalar.dma_start(out=negthr[:],
                        in_=vecs_ap[4:5, :].to_broadcast((128, P_CORE)))

    # ---------------- DVE: x cast + pads + horizontal taps ----------------
    xhs = []
    for j, (r0, nr) in enumerate(part_rows):
        xt = xts[j]
        xh = work_pool.tile([nr, B, W + 2 * RMAX], BF16, tag=f"xh_{j}")
        nc.vector.tensor_copy(out=xh[:, :, RMAX:RMAX + W],
                              in_=xt[:, :, RMAX:RMAX + W])
        nc.vector.tensor_copy(
            out=xh[:, :, 0:RMAX],
            in_=xh[:, :, RMAX:RMAX + 1].to_broadcast((nr, B, RMAX)))
        nc.vector.tensor_copy(
            out=xh[:, :, RMAX + W:],
            in_=xh[:, :, RMAX + W - 1:RMAX + W].to_broadcast((nr, B, RMAX)))
        xhs.append(xh)

    hs = {1: [], 2: [], 3: []}
    for j, (r0, nr) in enumerate(part_rows):
        xh = xhs[j]
        eng = nc.vector
        h1 = work_pool.tile([nr, B, W], BF16, tag=f"hs1_{j}")
        h2 = work_pool.tile([nr, B, W], BF16, tag=f"hs2_{j}")
        h3 = work_pool.tile([nr, B, W], BF16, tag=f"hs3_{j}")
        ta = work_pool.tile([nr, B, W], BF16, tag=f"hta_{j}")
        sl = lambda c: xh[:, :, c:c + W]
        eng.tensor_tensor(out=ta[:], in0=sl(2), in1=sl(3), op=Alu.add)
        eng.tensor_tensor(out=h1[:], in0=ta[:], in1=sl(4), op=Alu.add)
        eng.tensor_tensor(out=ta[:], in0=sl(1), in1=sl(5), op=Alu.add)
        eng.tensor_tensor(out=h2[:], in0=h1[:], in1=ta[:], op=Alu.add)
        eng.tensor_tensor(out=ta[:], in0=sl(0), in1=sl(6), op=Alu.add)
        eng.tensor_tensor(out=h3[:], in0=h2[:], in1=ta[:], op=Alu.add)
        hs[1].append(h1)
        hs[2].append(h2)
        hs[3].append(h3)

    # ---------------- Stage A (DVE, after taps): pair index prep ----------------
    # floor/clip all four offset rows at once: sfl [1, 4, P_CORE]
    #   rows: sy1, sx1, sy2, sx2 (vecs rows 0..3 are offy1, offx1, offy2, offx2)
    off4 = vt[0:1, 0:4, :]
    fi4 = const_pool.tile([1, 4, P_CORE], I32, tag="fi4")
    ff4 = const_pool.tile([1, 4, P_CORE], F32, tag="ff4")
    gt4 = const_pool.tile([1, 4, P_CORE], F32, tag="gt4")
    sfl = const_pool.tile([1, 4, P_CORE], F32, tag="sfl")
    nc.vector.tensor_copy(out=fi4[:], in_=off4)     # cast (round or trunc)
    nc.vector.tensor_copy(out=ff4[:], in_=fi4[:])   # back to f32, exact
    nc.vector.tensor_tensor(out=gt4[:], in0=ff4[:], in1=off4, op=Alu.is_gt)
    nc.vector.tensor_tensor(out=sfl[:], in0=ff4[:], in1=gt4[:], op=Alu.subtract)
    nc.vector.tensor_scalar_add(out=sfl[:], in0=sfl[:], scalar1=float(PAD))
    nc.vector.tensor_scalar(out=sfl[:], in0=sfl[:], scalar1=0.0,
                     scalar2=float(2 * PAD), op0=Alu.max, op1=Alu.min)
    sy1, sx1 = sfl[0:1, 0, :], sfl[0:1, 1, :]
    sy2, sx2 = sfl[0:1, 2, :], sfl[0:1, 3, :]
    syb = sfl[0:1, 0:3:2, :]  # both sy rows, stride 2

    # b = (sy>=16)+(sy>=32); t = sy - 16*b  (for both sy rows at once)
    bb = const_pool.tile([1, 2, P_CORE], F32, tag="bb")
    b32 = const_pool.tile([1, 2, P_CORE], F32, tag="b32")
    tt = const_pool.tile([1, 2, P_CORE], F32, tag="tt")
    nc.vector.tensor_scalar(out=bb[:], in0=syb, scalar1=16.0, scalar2=None, op0=Alu.is_ge)
    nc.vector.tensor_scalar(out=b32[:], in0=syb, scalar1=32.0, scalar2=None, op0=Alu.is_ge)
    nc.vector.tensor_tensor(out=bb[:], in0=bb[:], in1=b32[:], op=Alu.add)
    nc.vector.tensor_scalar(out=tt[:], in0=bb[:], scalar1=-16.0, scalar2=None, op0=Alu.mult)
    nc.vector.tensor_tensor(out=tt[:], in0=tt[:], in1=syb, op=Alu.add)

    radf = const_pool.tile([1, P_CORE], F32, tag="radf")
    nc.vector.tensor_copy(out=radf[:], in_=radii_t[:])
    nc.vector.tensor_scalar(out=radf[:], in0=radf[:], scalar1=1.0, scalar2=3.0,
                     op0=Alu.max, op1=Alu.min)
    dbase = const_pool.tile([1, P_CORE], F32, tag="dbase")
    nc.vector.tensor_scalar(out=dbase[:], in0=radf[:], scalar1=1.0, scalar2=512.0,
                     op0=Alu.subtract, op1=Alu.mult)

    # tab_pe [1, n_lgrp, LGRP, 2]: off_w = b_w*TFREE + (d-1)*512 + sx_w
    # tab_dve [1, n_lgrp, LGRP, 2]: e1 = 128 + t1,  e2 = EW + 128 + t2
    n_lgrp = P_CORE // LGRP
    tabf_pe = const_pool.tile([1, n_lgrp, LGRP, 2], F32, tag="tabf_pe")
    tabf_dve = const_pool.tile([1, n_lgrp, LGRP, 2], F32, tag="tabf_dve")
    tmp = const_pool.tile([1, P_CORE], F32, tag="tab_tmp")
    for w, sxw in enumerate((sx1, sx2)):
        nc.vector.tensor_scalar(out=tmp[:], in0=bb[0:1, w, :], scalar1=float(TFREE),
                         scalar2=None, op0=Alu.mult)
        nc.vector.tensor_tensor(out=tmp[:], in0=tmp[:], in1=dbase[:], op=Alu.add)
        nc.vector.tensor_tensor(
            out=tabf_pe[0:1, :, :, w].rearrange("a g i -> a (g i)"),
            in0=tmp[:], in1=sxw, op=Alu.add)
    nc.vector.tensor_scalar_add(
        out=tabf_dve[0:1, :, :, 0].rearrange("a g i -> a (g i)"),
        in0=tt[0:1, 0, :], scalar1=128.0)
    nc.vector.tensor_scalar_add(
        out=tabf_dve[0:1, :, :, 1].rearrange("a g i -> a (g i)"),
        in0=tt[0:1, 1, :], scalar1=float(EW + 128))
    tab_pe = const_pool.tile([1, n_lgrp, LGRP, 2], I32, tag="tab_pe")
    tab_dve = const_pool.tile([1, n_lgrp, LGRP, 2], I32, tag="tab_dve")
    nc.vector.tensor_copy(out=tab_pe[:], in_=tabf_pe[:])
    nc.vector.tensor_copy(out=tab_dve[:], in_=tabf_dve[:])

    # ---------------- Stage B: plane tiles (PE matmul + ACT evict) -------
    # plane tensor [128, NT, 3, B, HP] bf16; tile starts {0,16,32,112,128,144}.
    pl = work_pool.tile([128, NT, 3, B, HP], BF16, tag="pl")
    for d in (1, 2, 3):
        area = float((2 * d + 1) ** 2)
        for j, tix in ((0, 0), (1, 4)):
            ps = psumB_pool.tile([128, NB], F32, tag=f"bps{j}")
            sdt_t = sdt_lo if j == 0 else sdt_hi
            nc.tensor.matmul(out=ps[:], lhsT=sdt_t[:, d - 1, :],
                             rhs=hs[d][j][:].rearrange("r b w -> r (b w)"),
                             start=True, stop=True)
            nc.scalar.activation(pl[:, tix, d - 1, :, PAD:PAD + W],
                                 ps[:].rearrange("r (b w) -> r b w", b=B),
                                 Act.Copy, scale=1.0 / area)
            nc.vector.tensor_copy(
                out=pl[:, tix, d - 1, :, 0:PAD],
                in_=pl[:, tix, d - 1, :, PAD:PAD + 1].to_broadcast((128, B, PAD)))
            nc.vector.tensor_copy(
                out=pl[:, tix, d - 1, :, PAD + W:],
                in_=pl[:, tix, d - 1, :, PAD + W - 1:PAD + W].to_broadcast(
                    (128, B, PAD)))

    # warm-up matmuls: keep the PE busy (DVFS ramp) while the S->S plane
    # copies run; results are discarded.
    for wmm in range(N_WARM):
        wps = psumB_pool.tile([128, NB], F32, tag=f"bps{wmm % 2}")
        nc.tensor.matmul(out=wps[:], lhsT=sdt_lo[:, 0, :], rhs=e2t[:, 0:NB],
                         start=True, stop=True)

    # S->S partition-shifted copies (all d at once):
    # (dst_tile, src_tile, src_p0, n, dst_p0); tile5 rows 112.. are filler
    # (finite values only, never selected by E).
    for k, (dt_, st_, sp, n, dp) in enumerate((
            (1, 0, 16, 112, 0), (1, 4, 0, 16, 112),
            (2, 0, 32, 96, 0), (2, 4, 0, 32, 96),
            (3, 0, 112, 16, 0), (3, 4, 0, 112, 16),
            (5, 4, 16, 112, 0), (5, 4, 112, 16, 112))):
        eng = nc.sync if k % 2 == 0 else nc.scalar
        eng.dma_start(out=pl[dp:dp + n, dt_, :, :, :],
                      in_=pl[sp:sp + n, st_, :, :, :])

    # ---------------- Stage C: PE gather ----------------
    pl_base = pl[:].offset
    e2_base = e2t[:].offset
    assert isinstance(pl_base, int) and isinstance(e2_base, int)
    PL_AP = [[NT * TFREE, 128], [HP, B], [1, W]]   # [128, b, 224] window view
    MAX_RHS = 2 * TFREE + 2 * 512 + 2 * PAD

    def rhs_ap(off, extra):
        return bass.AP(pl[:].tensor, pl_base + extra + off,
                       [r[:] for r in PL_AP])

    def lhs_ap(off):
        return bass.AP(e2t[:].tensor, e2_base + off, [[2 * EW, 128], [1, HB]])

    n_ogrp = P_CORE // OGRP
    sts = {}
    for g in range(n_lgrp):
        _, pe_vals = nc.values_load_multi_w_load_instructions(
            tab_pe[0:1, g, :, :], engines=[EngT.PE],
            min_val=0, max_val=MAX_RHS, skip_runtime_bounds_check=True)
        _, dve_vals = nc.values_load_multi_w_load_instructions(
            tab_dve[0:1, g, :, :], engines=[EngT.DVE],
            min_val=0, max_val=EW + 128 + 16, skip_runtime_bounds_check=True)
        # stage all LGRP pairs' E slices up front so the PE never waits
        ess = []
        for i in range(LGRP):
            es1 = estage_pool.tile([128, HB], BF16, tag="es1")
            es2 = estage_pool.tile([128, HB], BF16, tag="es2")
            nc.vector.tensor_copy(out=es1[:], in_=lhs_ap(dve_vals[2 * i]))
            nc.vector.tensor_copy(out=es2[:], in_=lhs_ap(dve_vals[2 * i + 1]))
            ess.append((es1, es2))
        for i in range(LGRP):
            p = g * LGRP + i
            og, oi = p // OGRP, p % OGRP
            if oi == 0:
                st0 = stage_pool.tile([HB, OGRP, B, W], BF16, tag="st0")
                st1 = stage_pool.tile([HB, OGRP, B, W], BF16, tag="st1")
                sts[og] = (st0, st1)
            st0, st1 = sts[og]
            or1, or2 = pe_vals[2 * i], pe_vals[2 * i + 1]
            es1, es2 = ess[i]
            ps0 = psum_pool.tile([HB, NB], F32, tag="ps0")
            ps1 = psum_pool.tile([HB, NB], F32, tag="ps1")
            nc.tensor.matmul(out=ps0[:], lhsT=es1[:], rhs=rhs_ap(or1, 0),
                             start=True, stop=False)
            nc.tensor.matmul(out=ps1[:], lhsT=es1[:], rhs=rhs_ap(or1, D1),
                             start=True, stop=False)
            nc.tensor.matmul(out=ps0[:], lhsT=es2[:], rhs=rhs_ap(or2, 0),
                             start=False, stop=True)
            nc.tensor.matmul(out=ps1[:], lhsT=es2[:], rhs=rhs_ap(or2, D1),
                             start=False, stop=True)
            nc.scalar.activation(st0[:, oi, :, :],
                                 ps0[:].rearrange("r (b w) -> r b w", b=B),
                                 Act.Identity, bias=negthr[0:HB, p:p + 1])
            ps1_on_act = (i % 3 == 2) and g < n_lgrp - 1
            if ps1_on_act:
                nc.scalar.activation(st1[:, oi, :, :],
                                     ps1[:].rearrange("r (b w) -> r b w", b=B),
                                     Act.Identity, bias=negthr[0:HB, p:p + 1])
            else:
                nc.vector.tensor_scalar_add(
                    out=st1[:, oi, :, :],
                    in0=ps1[:].rearrange("r (b w) -> r b w", b=B),
                    scalar1=negthr[0:HB, p:p + 1])
            if oi == OGRP - 1:
                e0 = nc.sync if og % 2 == 0 else nc.scalar
                e1 = nc.scalar if og % 2 == 0 else nc.sync
                e0.dma_start(out=out0_ap[:, og * OGRP:(og + 1) * OGRP],
                             in_=st0[:])
                e1.dma_start(out=out1_ap[:, og * OGRP:(og + 1) * OGRP],
                             in_=st1[:])

    ctx.close()


_COMPILED = {}


def _get_compiled():
    if "nc" not in _COMPILED:
        nc = bacc.Bacc("TRN2", target_bir_lowering=False, debug=False,
                       num_devices=N_CORES)
        build_device_program(nc)
        nc.compile()
        _COMPILED["nc"] = nc
    return _COMPILED["nc"]


def _ensure_ntff_hook():
    """The agent image's antenv lacks axon_hooks; shim it so trace=True can
    drive NTFF profiling via the boot module's ctypes hook (test-only path)."""
    import types

    try:
        from antenv.axon_hooks import get_axon_ntff_profile_hook  # noqa: F401
        return
    except ImportError:
        pass
    import antenv

    mod = types.ModuleType("antenv.axon_hooks")
    _hook = [None]
    mod.set_axon_ntff_profile_hook = lambda h: _hook.__setitem__(0, h)
    mod.get_axon_ntff_profile_hook = lambda: _hook[0]
    sys.modules["antenv.axon_hooks"] = mod
    antenv.axon_hooks = mod
    from trn_agent_boot.trn_boot import _ntff_profile_via_ctypes

    mod.set_axon_ntff_profile_hook(
        _ntff_profile_via_ctypes("/opt/axon/libaxon_pjrt.so"))


def run(inputs: dict, trace: bool = False):
    """Run on the 8 cores. Returns (full output [B,256,H,W], exec_time_ns|None)."""
    x = np.asarray(inputs["x"], dtype=np.float32).reshape(B, H, W)
    offset_x1 = np.asarray(inputs["offset_x1"], np.float32)
    offset_x2 = np.asarray(inputs["offset_x2"], np.float32)
    offset_y1 = np.asarray(inputs["offset_y1"], np.float32)
    offset_y2 = np.asarray(inputs["offset_y2"], np.float32)
    radii = np.asarray(inputs["radii"]).astype(np.int32)
    thresholds = np.asarray(inputs["thresholds"], np.float32)

    sdt = _band_matrices()
    e2 = _shift_identity()
    nc = _get_compiled()

    in_maps = []
    for c in range(N_CORES):
        sl = slice(c * P_CORE, (c + 1) * P_CORE)
        vecs = np.stack([offset_y1[sl], offset_x1[sl], offset_y2[sl],
                         offset_x2[sl], -thresholds[sl]]).astype(np.float32)
        in_maps.append({
            "x": x,
            "vecs": vecs,
            "radii": radii[sl].reshape(1, P_CORE),
            "sdt": sdt,
            "e2": e2,
        })

    if trace:
        _ensure_ntff_hook()
    res = run_bass_kernel_spmd(nc, in_maps, list(range(N_CORES)), trace=trace)
    # per-core out0/out1 [112, P_CORE, B, W] bf16 (rows 0..111 / 112..223)
    full = np.empty((B, P_TOTAL, H, W), np.float32)
    for c in range(N_CORES):
        o0 = np.asarray(res.results[c]["out0"]).astype(np.float32)
        o1 = np.asarray(res.results[c]["out1"]).astype(np.float32)
        sl = slice(c * P_CORE, (c + 1) * P_CORE)
        full[:, sl, :HB, :] = o0.transpose(2, 1, 0, 3)
        full[:, sl, HB:, :] = o1.transpose(2, 1, 0, 3)
    return full, res.exec_time_ns


def kernel(x, offset_x1, offset_x2, offset_y1, offset_y2, radii, thresholds,
           max_radius):
    out, _ = run({
        "x": x, "offset_x1": offset_x1, "offset_x2": offset_x2,
        "offset_y1": offset_y1, "offset_y2": offset_y2,
        "radii": radii, "thresholds": thresholds, "max_radius": max_radius,
    })
    return out


if __name__ == "__main__":
    # smoke test with random data
    rng = np.random.default_rng(0)
    out = kernel(
        x=rng.standard_normal((B, 1, H, W), dtype=np.float32),
        offset_x1=rng.uniform(-16, 16, P_TOTAL).astype(np.float32),
        offset_x2=rng.uniform(-16, 16, P_TOTAL).astype(np.float32),
        offset_y1=rng.uniform(-16, 16, P_TOTAL).astype(np.float32),
        offset_y2=rng.uniform(-16, 16, P_TOTAL).astype(np.float32),
        radii=rng.integers(1, 4, P_TOTAL).astype(np.int32),
        thresholds=(rng.standard_normal(P_TOTAL) * 0.1).astype(np.float32),
        max_radius=3,
    )
    print("out", out.shape, out.dtype, float(np.abs(out).max()))
